# revision 8
# baseline (speedup 1.0000x reference)
"""CGCNN regressor on 8 trn2 NeuronCores.

Sharding: graphs 32/core -> contiguous node blocks; edges live on dst's core.
Per core, nodes are permuted into 52 "ranges" of 128 (degree-balanced bin
packing, <=512 edges/range); each range owns 4 edge chunks of 128 slots.
Per layer: h (fp16, scaled by HSC) is AllGathered to a replicated pair-table
[26624, 256]; h[src] is fetched with one dma_gather(transpose=True) per block
(the pair trick keeps indices < 32768 int16) and the even/odd half is merged
in place with copy_predicated. Messages are computed as fp16/bf16 matmuls in
natural layout [slots, 256] with everything scaled by HSC so intermediates
fit fp16: src term via hi/lo bf16 weight pairs, edge_attr term via fp16
edge features and hi/lo bf16 weights, and the dst term via a precomputed
one-hot (sscT) against hi/lo fp16 dst projections. Per gather block (13
chunks), sigmoid/softplus run as 3 large activations (sigmoid on both
halves, then Ln for a stable softplus), so activation-table reloads happen
twice per block instead of per chunk. Aggregation is a one-hot matmul into
[range,128] PSUM. Pool/head run on 32 graphs/core; host concatenates the
8x[32] outputs.
"""

import os
import sys

import numpy as np

try:
    import concourse.bass as bass
except ImportError:  # grading env fallback
    sys.path.insert(0, "/opt/trn_rl_repo")
    import concourse.bass as bass

import concourse.mybir as mybir
import concourse.tile as tile
from concourse import bacc
from concourse.bass_utils import run_bass_kernel_spmd

try:
    import ml_dtypes

    BF16 = ml_dtypes.bfloat16
    F16 = np.float16
except ImportError:
    BF16 = mybir.dt.np(mybir.dt.bfloat16)
    F16 = np.float16

F32 = np.float32

# problem constants
N, E, H, ED, NG, NEMB, L = 50000, 200000, 128, 50, 256, 100, 6
C = 8               # cores
GPC = NG // C       # graphs per core
NT = 52             # node tiles (ranges) per core
N_LOC = NT * 128    # padded local nodes (6656)
CPR = 4             # chunks per range
NCHUNK = NT * CPR   # 208
NSLOT = NCHUNK * 128  # 26624 edge slots
CPB = 13            # chunks per gather block
NBLK = NCHUNK // CPB  # 16
SLOT_B = CPB * 128  # 1664 slots per block
PAIRS = C * N_LOC // 2  # 26624 pair rows in the replicated h table
HSC = 1.0 / 16.0    # fp16 scale: h table, fs, p_fs, msg all carry HSC
SIGC = 6.1e-5       # sigmoid clamp before Ln (fp16 min normal)

_L_RUN = int(os.environ.get("KERNEL_LAYERS", str(L)))
_PHASE = int(os.environ.get("KERNEL_PHASE", "99"))  # 1=proj 2=+conv 99=all


# ---------------------------------------------------------------------------
# host-side preprocessing
# ---------------------------------------------------------------------------

def _wrap16(idx, pad_to):
    """int16 index tensor in dma_gather layout: [128, pad_to//16],
    slot i -> row i%16, col i//16; replicated 8x down the partitions."""
    a = np.full(pad_to, 0, np.int16)
    a[: len(idx)] = idx.astype(np.int16)
    w = a.reshape(pad_to // 16, 16).T  # [16, pad/16]
    return np.tile(w, (8, 1)).copy()


def _bn_fold(p, bias=None):
    gamma, beta, mean, var = [np.asarray(x, np.float64) for x in p]
    scale = gamma / np.sqrt(var + 1e-5)
    shift = beta - mean * scale
    if bias is not None:
        shift = shift + np.asarray(bias, np.float64) * scale
    return scale.astype(F32), shift.astype(F32)


def _rep(row, parts=128):
    row = np.asarray(row, F32).reshape(1, -1)
    return np.repeat(row, parts, axis=0).copy()


def _hilo(x, dt):
    x = np.asarray(x, F32)
    hi = x.astype(dt)
    lo = (x - hi.astype(F32)).astype(dt)
    return hi, lo


def _prep(inputs):
    x_atom = np.asarray(inputs["x_atom"]).astype(np.int64)
    ei = np.asarray(inputs["edge_index"]).astype(np.int64)
    ea = np.asarray(inputs["edge_attr"]).astype(F32)
    batch = np.asarray(inputs["batch"]).astype(np.int64)
    src, dst = ei[0], ei[1]

    node_start = np.searchsorted(batch, np.arange(0, NG + 1, GPC))
    deg = np.bincount(dst, minlength=N)

    # global node -> (core, local id); degree-balanced FFD into NT ranges/core
    lid = np.empty(N, np.int64)
    core_of = np.empty(N, np.int64)
    for c in range(C):
        s, e = node_start[c], node_start[c + 1]
        nodes = np.arange(s, e)
        assert len(nodes) <= N_LOC, f"core {c}: {len(nodes)} > {N_LOC}"
        order = nodes[np.argsort(-deg[nodes], kind="stable")]
        cap_n = np.full(NT, 128, np.int64)
        cap_e = np.full(NT, CPR * 128, np.int64)
        pos = np.zeros(NT, np.int64)
        for g in order:
            d = deg[g]
            cand = np.where((cap_n > 0) & (cap_e >= d))[0]
            assert len(cand), f"core {c}: range packing failed (deg {d})"
            r = cand[np.argmax(cap_e[cand])]
            lid[g] = r * 128 + pos[r]
            pos[r] += 1
            cap_n[r] -= 1
            cap_e[r] -= d
        core_of[s:e] = c

    gaddr = core_of * N_LOC + lid  # global address in the replicated table

    in_maps = []
    for c in range(C):
        s, e = node_start[c], node_start[c + 1]
        slot_pair = np.zeros(NSLOT, np.int64)
        slot_par = np.zeros(NSLOT, np.uint8)
        slot_dst = np.full(NSLOT, -1, np.int64)  # -1 = pad slot
        slot_ea = np.zeros((NSLOT, ED), F32)
        slot_bias = np.zeros(NSLOT, F32)

        emask = (dst >= s) & (dst < e)
        ce_src, ce_dst, ce_ea = src[emask], dst[emask], ea[emask]
        r_of_e = lid[ce_dst] // 128
        for r in range(NT):
            sel = np.where(r_of_e == r)[0]
            assert len(sel) <= CPR * 128, f"core {c} range {r}: {len(sel)}"
            base = r * CPR * 128
            sl = base + np.arange(len(sel))
            ga = gaddr[ce_src[sel]]
            slot_pair[sl] = ga >> 1
            slot_par[sl] = (ga & 1).astype(np.uint8)
            slot_dst[sl] = lid[ce_dst[sel]] - r * 128
            slot_ea[sl] = ce_ea[sel]
            slot_bias[sl] = 1.0

        # precomputed one-hots: per chunk c, cols [0:128] = ssc (slot ->
        # dst row one-hot, partition = slot), cols [128:256] = sscT
        # (partition = dst row, col = slot)
        ssch = np.zeros((128, NCHUNK, 256), F16)
        sd = slot_dst.reshape(NCHUNK, 128)
        for ch in range(NCHUNK):
            valid = np.where(sd[ch] >= 0)[0]
            dcol = sd[ch][valid]
            ssch[valid, ch, dcol] = 1.0
            ssch[dcol, ch, 128 + valid] = 1.0

        # graph one-hot for pooling over local (permuted) node layout
        goh = np.zeros((128, NT * GPC), F32)
        xa_local = np.zeros(N_LOC, np.int64)
        nodes = np.arange(s, e)
        li = lid[nodes]
        xa_local[li] = x_atom[nodes]
        t_i, p_i = li // 128, li % 128
        goh[p_i, t_i * GPC + (batch[nodes] - c * GPC)] = 1.0

        goh2 = np.zeros((GPC, N_LOC), F32)
        goh2[batch[nodes] - c * GPC, li] = 1.0

        m = {
            "gidx": _wrap16(slot_pair, NSLOT),
            "xidx": _wrap16(xa_local, N_LOC),
            "pmask": np.repeat(
                slot_par.reshape(1, -1), 128, axis=0
            ).astype(np.uint8),
            "eaT": np.concatenate(
                [slot_ea.T, slot_bias.reshape(1, -1)], axis=0
            ).astype(F16),
            "ssch": ssch.reshape(128, NCHUNK * 256),
            "goh": goh,
            "goh2": goh2,
            "maskbias": ((goh - 1.0) * 1e30).astype(F32),
        }
        in_maps.append(m)

    # shared parameters
    conv_Wf = np.asarray(inputs["conv_Wf"], F32)
    conv_Ws = np.asarray(inputs["conv_Ws"], F32)
    conv_bf = np.asarray(inputs["conv_bf"], F32)
    conv_bs = np.asarray(inputs["conv_bs"], F32)
    conv_bn = np.asarray(inputs["conv_bn"], F32)

    # all message terms carry the HSC scale: fs_s = fs * HSC
    # src term: merged (= h*HSC) @ Wsrc  -> weights plain
    wsrc = np.concatenate(
        [
            np.concatenate([conv_Wf[l, H : 2 * H], conv_Ws[l, H : 2 * H]], 1)
            for l in range(L)
        ],
        axis=1,
    )  # [128, L*256]
    # dst term: p_fs = (h @ Wdst) * HSC -> fold HSC into Wdst
    wdst = np.concatenate(
        [
            np.concatenate([conv_Wf[l, :H], conv_Ws[l, :H]], 1)
            for l in range(L)
        ],
        axis=1,
    ) * HSC
    # edge term: ea @ (Wea * HSC) (+ bias * HSC)
    wea = np.concatenate(
        [
            np.concatenate(
                [
                    np.concatenate([conv_Wf[l, 2 * H :], conv_Ws[l, 2 * H :]], 1),
                    np.concatenate([conv_bf[l], conv_bs[l]]).reshape(1, -1),
                ],
                axis=0,
            )
            for l in range(L)
        ],
        axis=1,
    ) * HSC  # [51, L*256]
    wsrc_hi, wsrc_lo = _hilo(wsrc, BF16)
    wea_hi, wea_lo = _hilo(wea, BF16)
    convss = np.concatenate(
        [
            np.concatenate([_rep(sc), _rep(sh)], axis=1)
            for sc, sh in ((_bn_fold(conv_bn[l])) for l in range(L))
        ],
        axis=1,
    )  # [128, L*256]

    psc, psh = _bn_fold(np.asarray(inputs["proj_bn"], F32),
                        bias=np.asarray(inputs["proj_b"], F32))
    h1sc, h1sh = _bn_fold(np.asarray(inputs["head_bn1"], F32),
                          bias=np.asarray(inputs["head_b1"], F32))
    h2sc, h2sh = _bn_fold(np.asarray(inputs["head_bn2"], F32),
                          bias=np.asarray(inputs["head_b2"], F32))

    shared = {
        "emb": np.asarray(inputs["emb"], F32),
        "projW": np.asarray(inputs["proj_W"], F32),
        "projss": np.concatenate([_rep(psc), _rep(psh)], axis=1),
        "wsrchi": wsrc_hi,
        "wsrclo": wsrc_lo,
        "wdst": wdst.astype(F32),
        "weahi": wea_hi,
        "wealo": wea_lo,
        "convss": convss,
        "gatew1": np.asarray(inputs["gate_W1"], F32),
        "gateb1": _rep(np.asarray(inputs["gate_b1"], F32)),
        "gatew2": np.asarray(inputs["gate_W2"], F32),
        "gateb2": _rep(np.asarray(inputs["gate_b2"], F32).reshape(1)),
        "headw1": np.asarray(inputs["head_W1"], F32),
        "h1ss": np.concatenate([_rep(h1sc), _rep(h1sh)], axis=1),
        "headw2": np.asarray(inputs["head_W2"], F32),
        "h2ss": np.concatenate([_rep(h2sc), _rep(h2sh)], axis=1),
        "headw3": np.asarray(inputs["head_W3"], F32),
        "h3b": _rep(np.asarray(inputs["head_b3"], F32)),
        "headw4": np.asarray(inputs["head_W4"], F32),
        "h4b": _rep(np.asarray(inputs["head_b4"], F32).reshape(1)),
        "identf": np.eye(128, dtype=F32),
    }
    for m in in_maps:
        m.update(shared)
    return in_maps


# ---------------------------------------------------------------------------
# bass program
# ---------------------------------------------------------------------------

def _build():
    dt = mybir.dt
    nc = bacc.Bacc(num_devices=C)

    def par(name, shape, dtp):
        return nc.declare_dram_parameter(name, list(shape), dtp, isOutput=False)

    gidx_d = par("gidx", [128, NSLOT // 16], dt.int16)
    xidx_d = par("xidx", [128, N_LOC // 16], dt.int16)
    pmask_d = par("pmask", [128, NSLOT], dt.uint8)
    eaT_d = par("eaT", [ED + 1, NSLOT], dt.float16)
    ssch_d = par("ssch", [128, NCHUNK * 256], dt.float16)
    goh_d = par("goh", [128, NT * GPC], dt.float32)
    goh2_d = par("goh2", [GPC, N_LOC], dt.float32)
    maskbias_d = par("maskbias", [128, NT * GPC], dt.float32)
    emb_d = par("emb", [NEMB, H], dt.float32)
    projW_d = par("projW", [H, H], dt.float32)
    projss_d = par("projss", [128, 256], dt.float32)
    wsrchi_d = par("wsrchi", [H, L * 256], dt.bfloat16)
    wsrclo_d = par("wsrclo", [H, L * 256], dt.bfloat16)
    wdst_d = par("wdst", [H, L * 256], dt.float32)
    weahi_d = par("weahi", [ED + 1, L * 256], dt.bfloat16)
    wealo_d = par("wealo", [ED + 1, L * 256], dt.bfloat16)
    convss_d = par("convss", [128, L * 256], dt.float32)
    gatew1_d = par("gatew1", [H, H // 2], dt.float32)
    gateb1_d = par("gateb1", [128, H // 2], dt.float32)
    gatew2_d = par("gatew2", [H // 2, 1], dt.float32)
    gateb2_d = par("gateb2", [128, 1], dt.float32)
    headw1_d = par("headw1", [H, H], dt.float32)
    h1ss_d = par("h1ss", [128, 256], dt.float32)
    headw2_d = par("headw2", [H, H // 2], dt.float32)
    h2ss_d = par("h2ss", [128, 128], dt.float32)
    headw3_d = par("headw3", [H // 2, H // 4], dt.float32)
    h3b_d = par("h3b", [128, H // 4], dt.float32)
    headw4_d = par("headw4", [H // 4, 1], dt.float32)
    h4b_d = par("h4b", [128, 1], dt.float32)
    identf_d = par("identf", [128, 128], dt.float32)

    out_d = nc.declare_dram_parameter("out", [GPC, 1], dt.float32, isOutput=True)

    hstage = nc.dram_tensor("hstage", [N_LOC // 2, 256], dt.float16)
    hfull = [
        nc.dram_tensor(f"hfull{i}", [PAIRS, 256], dt.float16,
                       addr_space="Shared")
        for i in range(2)
    ]

    FT, F16T = dt.float32, dt.float16
    AF = mybir.ActivationFunctionType
    OP = mybir.AluOpType

    with tile.TileContext(nc) as tc:
        with (
            tc.tile_pool(name="const", bufs=1) as cpool,
            tc.tile_pool(name="state", bufs=1) as spool,
            tc.tile_pool(name="psA", bufs=2, space="PSUM") as psA,   # [128,256] fs
            tc.tile_pool(name="psB", bufs=2, space="PSUM") as psB,   # [128,256] p/head
            tc.tile_pool(name="psT", bufs=2, space="PSUM") as psT,   # transposes
            tc.tile_pool(name="psG", bufs=2, space="PSUM") as psG,   # aggr
        ):
            # ---------------- resident tiles ----------------
            def load(pool, dram, shape, dtp):
                nm = f"c_{dram.name}"
                t = pool.tile(shape, dtp, name=nm, tag=nm)
                nc.sync.dma_start(out=t[:], in_=dram[:])
                return t

            gidx_t = load(cpool, gidx_d, [128, NSLOT // 16], dt.int16)
            projW_t = load(cpool, projW_d, [H, H], FT)
            projss_t = load(cpool, projss_d, [128, 256], FT)
            wsrchi_t = load(cpool, wsrchi_d, [H, L * 256], dt.bfloat16)
            wsrclo_t = load(cpool, wsrclo_d, [H, L * 256], dt.bfloat16)
            wdst_t = load(cpool, wdst_d, [H, L * 256], FT)
            weahi_t = load(cpool, weahi_d, [ED + 1, L * 256], dt.bfloat16)
            wealo_t = load(cpool, wealo_d, [ED + 1, L * 256], dt.bfloat16)
            convss_t = load(cpool, convss_d, [128, L * 256], FT)
            identf_t = load(cpool, identf_d, [128, 128], FT)

            h_loc = spool.tile([128, NT, H], FT, tag="h_loc")
            pfs_hi = spool.tile([128, NT, 256], F16T, tag="pfs_hi")
            pfs_lo = spool.tile([128, NT, 256], F16T, tag="pfs_lo")
            h_bf = spool.tile([128, NT, H], F16T, tag="h_bf")

            # ---------------- embedding + projection ----------------
            with (
                tc.tile_pool(name="proj", bufs=2) as prpool,
                tc.tile_pool(name="projc", bufs=1) as prcpool,
            ):
                xidx_t = load(prcpool, xidx_d, [128, N_LOC // 16], dt.int16)
                TPG = 13  # node tiles per gather call
                for g in range(NT // TPG):
                    h0 = prpool.tile([128, TPG, H], FT, tag="h0")
                    nc.gpsimd.dma_gather(
                        h0[:], emb_d[:],
                        xidx_t[:, g * (TPG * 8) : (g + 1) * (TPG * 8)],
                        TPG * 128, TPG * 128, H, single_packet=False,
                    )
                    for tt in range(TPG):
                        t = g * TPG + tt
                        pT = psT.tile([128, 128], FT, tag="tr", name=f"prT{t}")
                        nc.tensor.transpose(pT[:], h0[:, tt, :], identf_t[:])
                        hT = prpool.tile([128, 128], FT, tag="hT32",
                                         name=f"prh{t}")
                        nc.vector.tensor_copy(hT[:], pT[:])
                        pm = psB.tile([128, 256], FT, tag="pB", name=f"prm{t}")
                        nc.tensor.matmul(pm[:, :H], hT[:], projW_t[:],
                                         start=True, stop=True)
                        t1 = prpool.tile([128, H], FT, tag="nupd",
                                         name=f"pru{t}")
                        nc.vector.tensor_tensor(
                            out=t1[:], in0=pm[:, :H], in1=projss_t[:, :128],
                            op=OP.mult)
                        nc.vector.tensor_tensor(
                            out=t1[:], in0=t1[:], in1=projss_t[:, 128:],
                            op=OP.add)
                        sgp = prpool.tile([128, H], FT, tag="sgp",
                                          name=f"prs{t}")
                        nc.scalar.activation(sgp[:], t1[:], AF.Sigmoid)
                        nc.vector.tensor_mul(out=h_loc[:, t, :], in0=t1[:],
                                             in1=sgp[:])

            if _PHASE <= 1:
                dbg = spool.tile([GPC, 1], FT, tag="dbg", name="dbg1")
                nc.vector.tensor_copy(dbg[:], h_loc[:GPC, 0, 0:1])
                nc.sync.dma_start(out=out_d[:], in_=dbg[:])

            # ---------------- conv layers ----------------
            with (
                tc.tile_pool(name="gbuf", bufs=2) as gpool,
                tc.tile_pool(name="work", bufs=2) as wpool,
                tc.tile_pool(name="wk1", bufs=1) as w1pool,
                tc.tile_pool(name="small", bufs=3) as smpool,
            ):
                for l in range(_L_RUN if _PHASE >= 2 else 0):
                    hf = hfull[l % 2]
                    # stage h as scaled fp16 + allgather
                    nc.vector.tensor_scalar_mul(
                        out=h_bf[:].rearrange("p t h -> p (t h)"),
                        in0=h_loc[:].rearrange("p t h -> p (t h)"),
                        scalar1=HSC)
                    nc.sync.dma_start(
                        out=hstage[:].rearrange("n (two h) -> (n two) h", two=2)
                        .rearrange("(t p) h -> p t h", p=128),
                        in_=h_bf[:],
                    )
                    nc.gpsimd.collective_compute(
                        "AllGather",
                        mybir.AluOpType.bypass,
                        replica_groups=[list(range(C))],
                        ins=[hstage[:]],
                        outs=[hf[:]],
                    )

                    # dst-side node projections pfs = (h @ Wdst)*HSC, hi/lo
                    for t in range(NT):
                        pT = psT.tile([128, 128], FT, tag="tr",
                                      name=f"pT_{l}_{t}")
                        nc.tensor.transpose(pT[:], h_loc[:, t, :], identf_t[:])
                        hTb = wpool.tile([128, 128], FT, tag="hTb",
                                         name=f"hTb_{l}_{t}")
                        nc.vector.tensor_copy(hTb[:], pT[:])
                        pm = psB.tile([128, 256], FT, tag="pB",
                                      name=f"pm_{l}_{t}")
                        nc.tensor.matmul(
                            pm[:], hTb[:], wdst_t[:, l * 256 : (l + 1) * 256],
                            start=True, stop=True)
                        nc.vector.tensor_copy(pfs_hi[:, t, :], pm[:])
                        nc.vector.tensor_tensor(
                            out=pfs_lo[:, t, :], in0=pm[:],
                            in1=pfs_hi[:, t, :], op=OP.subtract)

                    aggr = {}
                    for b in range(NBLK):
                        bsl = slice(b * SLOT_B, (b + 1) * SLOT_B)
                        gb = gpool.tile([128, 2, SLOT_B], F16T, tag="gb",
                                        name=f"gb_{l}_{b}")
                        nc.gpsimd.dma_gather(
                            gb[:], hf[:],
                            gidx_t[:, b * (SLOT_B // 16) : (b + 1) * (SLOT_B // 16)],
                            SLOT_B, SLOT_B, 256, transpose=True,
                            single_packet=False,
                        )
                        mask = wpool.tile([128, SLOT_B], dt.uint8, tag="mask",
                                          name=f"mk_{l}_{b}")
                        nc.sync.dma_start(out=mask[:], in_=pmask_d[:, bsl])
                        # merge even/odd half in place
                        nc.vector.copy_predicated(gb[:, 0, :], mask[:],
                                                  gb[:, 1, :])
                        ea_t = wpool.tile([ED + 1, SLOT_B], F16T, tag="ea",
                                          name=f"ea_{l}_{b}")
                        nc.sync.dma_start(out=ea_t[:], in_=eaT_d[:, bsl])
                        ssk = wpool.tile([128, CPB, 256], F16T, tag="ssk",
                                         name=f"ssk_{l}_{b}")
                        nc.sync.dma_start(
                            out=ssk[:],
                            in_=ssch_d[:, b * CPB * 256 : (b + 1) * CPB * 256])

                        # fs_s = fs*HSC accumulated per chunk; stashed fp16
                        # as [128, 2(half), CPB, 128]
                        fsacc = wpool.tile([128, 2, CPB, 128], F16T,
                                           tag="fsacc", name=f"fsa_{l}_{b}")
                        for j in range(CPB):
                            c = b * CPB + j
                            r = c // CPR
                            sl = slice(j * 128, (j + 1) * 128)
                            fs = psA.tile([128, 256], FT, tag="fs",
                                          name=f"fs_{l}_{c}")
                            lsl = slice(l * 256, (l + 1) * 256)
                            nc.tensor.matmul(fs[:], gb[:, 0, sl],
                                             wsrchi_t[:, lsl],
                                             start=True, stop=False)
                            nc.tensor.matmul(fs[:], gb[:, 0, sl],
                                             wsrclo_t[:, lsl],
                                             start=False, stop=False)
                            nc.tensor.matmul(fs[:], ea_t[:, sl],
                                             weahi_t[:, lsl],
                                             start=False, stop=False)
                            nc.tensor.matmul(fs[:], ea_t[:, sl],
                                             wealo_t[:, lsl],
                                             start=False, stop=False)
                            nc.tensor.matmul(fs[:], ssk[:, j, 128:256],
                                             pfs_hi[:, r, :],
                                             start=False, stop=False)
                            nc.tensor.matmul(fs[:], ssk[:, j, 128:256],
                                             pfs_lo[:, r, :],
                                             start=False, stop=True)
                            nc.vector.tensor_copy(
                                fsacc[:, :, j, :],
                                fs[:].rearrange("p (two h) -> p two h",
                                                two=2))

                        # block activations: sigmoid(f), sigmoid(-s), Ln
                        sgf = w1pool.tile([128, CPB, 128], F16T, tag="sgf",
                                         name=f"sgf_{l}_{b}")
                        nc.scalar.activation(
                            sgf[:].rearrange("p c h -> p (c h)"),
                            fsacc[:, 0, :, :].rearrange("p c h -> p (c h)"),
                            AF.Sigmoid, scale=1.0 / HSC)
                        sgc = wpool.tile([128, CPB, 128], F16T, tag="sgc",
                                         name=f"sgc_{l}_{b}")
                        nc.scalar.activation(
                            sgc[:].rearrange("p c h -> p (c h)"),
                            fsacc[:, 1, :, :].rearrange("p c h -> p (c h)"),
                            AF.Sigmoid, scale=-1.0 / HSC)
                        spc = w1pool.tile([128, CPB, 128], F16T, tag="spc",
                                         name=f"spc_{l}_{b}")
                        nc.vector.tensor_scalar_max(
                            out=spc[:].rearrange("p c h -> p (c h)"),
                            in0=sgc[:].rearrange("p c h -> p (c h)"),
                            scalar1=SIGC)
                        lnv = w1pool.tile([128, CPB, 128], FT, tag="lnv",
                                         name=f"lnv_{l}_{b}")
                        nc.scalar.activation(
                            lnv[:].rearrange("p c h -> p (c h)"),
                            spc[:].rearrange("p c h -> p (c h)"), AF.Ln)
                        # sp_s = max(-ln(sigc)*HSC, s_s)
                        spv = wpool.tile([128, CPB, 128], F16T, tag="spv",
                                         name=f"spv_{l}_{b}")
                        nc.vector.scalar_tensor_tensor(
                            out=spv[:].rearrange("p c h -> p (c h)"),
                            in0=lnv[:].rearrange("p c h -> p (c h)"),
                            scalar=-HSC,
                            in1=fsacc[:, 1, :, :].rearrange("p c h -> p (c h)"),
                            op0=OP.mult, op1=OP.max)
                        msgb = wpool.tile([128, CPB, 128], F16T, tag="msgb",
                                          name=f"msg_{l}_{b}")
                        nc.vector.tensor_mul(
                            out=msgb[:].rearrange("p c h -> p (c h)"),
                            in0=sgf[:].rearrange("p c h -> p (c h)"),
                            in1=spv[:].rearrange("p c h -> p (c h)"))

                        for j in range(CPB):
                            c = b * CPB + j
                            r = c // CPR
                            if c % CPR == 0:
                                aggr[r] = psG.tile([128, H], FT, tag="aggr",
                                                   name=f"aggr_{l}_{r}")
                            nc.tensor.matmul(
                                aggr[r][:], ssk[:, j, 0:128], msgb[:, j, :],
                                start=(c % CPR == 0),
                                stop=(c % CPR == CPR - 1))
                            if c % CPR == CPR - 1:
                                lss = convss_t[:, l * 256 : (l + 1) * 256]
                                u = smpool.tile([128, H], FT, tag="nupd",
                                                name=f"u_{l}_{r}")
                                nc.vector.scalar_tensor_tensor(
                                    out=u[:], in0=aggr[r][:],
                                    scalar=1.0 / HSC,
                                    in1=h_loc[:, r, :],
                                    op0=OP.mult, op1=OP.add)
                                nc.vector.tensor_tensor(
                                    out=u[:], in0=u[:], in1=lss[:, :128],
                                    op=OP.mult)
                                nc.vector.tensor_tensor(
                                    out=u[:], in0=u[:], in1=lss[:, 128:],
                                    op=OP.add)
                                us = smpool.tile([128, H], FT, tag="nsig",
                                                 name=f"us_{l}_{r}")
                                nc.scalar.activation(us[:], u[:], AF.Sigmoid)
                                nc.vector.tensor_mul(out=us[:], in0=u[:],
                                                     in1=us[:])
                                nc.vector.tensor_tensor(
                                    out=h_loc[:, r, :], in0=us[:],
                                    in1=h_loc[:, r, :], op=OP.add)
                                del aggr[r]

            if _PHASE == 2:
                dbg2 = spool.tile([GPC, 1], FT, tag="dbg", name="dbg2")
                nc.vector.tensor_copy(dbg2[:], h_loc[:GPC, 0, 0:1])
                nc.sync.dma_start(out=out_d[:], in_=dbg2[:])

            # ---------------- gate + pooling + head ----------------
            with (
                tc.tile_pool(name="poolc", bufs=1) as pcpool,
                tc.tile_pool(name="pools", bufs=3) as smpool,
            ):
              if _PHASE >= 5:
                goh_t = load(pcpool, goh_d, [128, NT * GPC], FT)
                goh2_t = load(pcpool, goh2_d, [GPC, N_LOC], FT)
                maskb_t = load(pcpool, maskbias_d, [128, NT * GPC], FT)
                gatew1_t = load(pcpool, gatew1_d, [H, H // 2], FT)
                gateb1_t = load(pcpool, gateb1_d, [128, H // 2], FT)
                gatew2_t = load(pcpool, gatew2_d, [H // 2, 1], FT)
                gateb2_t = load(pcpool, gateb2_d, [128, 1], FT)
                headw1_t = load(pcpool, headw1_d, [H, H], FT)
                h1ss_t = load(pcpool, h1ss_d, [128, 256], FT)
                headw2_t = load(pcpool, headw2_d, [H, H // 2], FT)
                h2ss_t = load(pcpool, h2ss_d, [128, 128], FT)
                headw3_t = load(pcpool, headw3_d, [H // 2, H // 4], FT)
                h3b_t = load(pcpool, h3b_d, [128, H // 4], FT)
                headw4_t = load(pcpool, headw4_d, [H // 4, 1], FT)
                h4b_t = load(pcpool, h4b_d, [128, 1], FT)

                g_all = pcpool.tile([128, NT], FT, name="g_all", tag="g_all")
                runmax = pcpool.tile([128, GPC], FT, name="runmax",
                                     tag="runmax")

                # pass 1: per-node gate scores g + running per-graph max
                for t in range(NT):
                    pT = psT.tile([128, 128], FT, tag="tr", name=f"gT{t}")
                    nc.tensor.transpose(pT[:], h_loc[:, t, :], identf_t[:])
                    hT = smpool.tile([128, 128], FT, tag="hT32",
                                     name=f"gh{t}")
                    nc.vector.tensor_copy(hT[:], pT[:])
                    g1 = psB.tile([128, 256], FT, tag="pB", name=f"g1_{t}")
                    nc.tensor.matmul(g1[:, : H // 2], hT[:], gatew1_t[:],
                                     start=True, stop=True)
                    s1 = smpool.tile([128, H // 2], FT, tag="s1",
                                     name=f"s1_{t}")
                    nc.vector.tensor_tensor(
                        out=s1[:], in0=g1[:, : H // 2], in1=gateb1_t[:],
                        op=OP.add)
                    s1s = smpool.tile([128, H // 2], FT, tag="s1s",
                                      name=f"s1s_{t}")
                    nc.scalar.activation(s1s[:], s1[:], AF.Sigmoid)
                    nc.vector.tensor_mul(out=s1[:], in0=s1[:], in1=s1s[:])
                    pT2 = psT.tile([128, 128], FT, tag="tr", name=f"gU{t}")
                    nc.tensor.transpose(pT2[: H // 2, :], s1[:], identf_t[:])
                    s1T = smpool.tile([H // 2, 128], FT, tag="s1T",
                                      name=f"s1T_{t}")
                    nc.vector.tensor_copy(s1T[:], pT2[: H // 2, :])
                    g2 = psT.tile([128, 128], FT, tag="tr", name=f"g2_{t}")
                    nc.tensor.matmul(g2[:, :1], s1T[:], gatew2_t[:],
                                     start=True, stop=True)
                    nc.vector.tensor_tensor(
                        out=g_all[:, t : t + 1], in0=g2[:, :1],
                        in1=gateb2_t[:], op=OP.add)
                    gm = smpool.tile([128, GPC], FT, tag="gm",
                                     name=f"gm_{t}")
                    nc.vector.tensor_tensor(
                        out=gm[:],
                        in0=g_all[:, t : t + 1].to_broadcast([128, GPC]),
                        in1=goh_t[:, t * GPC : (t + 1) * GPC], op=OP.mult)
                    nc.vector.tensor_tensor(
                        out=gm[:], in0=gm[:],
                        in1=maskb_t[:, t * GPC : (t + 1) * GPC], op=OP.add)
                    if t == 0:
                        nc.vector.tensor_copy(runmax[:], gm[:])
                    else:
                        nc.vector.tensor_max(out=runmax[:], in0=runmax[:],
                                             in1=gm[:])

                # reduce running max across partitions -> gmax [GPC, 1]
                pTm = psT.tile([128, 128], FT, tag="tr", name="pTm")
                nc.tensor.transpose(pTm[:GPC, :], runmax[:], identf_t[:])
                rmT = smpool.tile([GPC, 128], FT, tag="rmT", name="rmT")
                nc.vector.tensor_copy(rmT[:], pTm[:GPC, :])
                negmax = smpool.tile([GPC, 1], FT, tag="negmax",
                                     name="negmax")
                nc.vector.tensor_reduce(out=negmax[:], in_=rmT[:],
                                        axis=mybir.AxisListType.X,
                                        op=OP.max)
                nc.vector.tensor_scalar_mul(out=negmax[:], in0=negmax[:],
                                            scalar1=-1.0)

                # pass 2: e = exp(min(g - gmax[graph], 20)), pooled sums
                pool_ps = psA.tile([GPC, H + 1], FT, tag="fs", name="pool_ps")
                for t in range(NT):
                    nK = psT.tile([128, 128], FT, tag="tr", name=f"nK{t}")
                    nc.tensor.matmul(
                        nK[:, :1], goh2_t[:, t * 128 : (t + 1) * 128],
                        negmax[:], start=True, stop=True)
                    earg = smpool.tile([128, 1], FT, tag="earg",
                                       name=f"ea2_{t}")
                    nc.vector.tensor_tensor(
                        out=earg[:], in0=g_all[:, t : t + 1], in1=nK[:, :1],
                        op=OP.add)
                    nc.vector.tensor_scalar_min(out=earg[:], in0=earg[:],
                                                scalar1=20.0)
                    ecol = smpool.tile([128, 1], FT, tag="ecol",
                                       name=f"ec_{t}")
                    nc.scalar.activation(ecol[:], earg[:], AF.Exp)
                    rhs = smpool.tile([128, H + 1], FT, tag="rhs",
                                      name=f"rhs_{t}")
                    nc.vector.tensor_scalar(
                        out=rhs[:, :H], in0=h_loc[:, t, :], scalar1=ecol[:],
                        scalar2=None, op0=OP.mult)
                    nc.vector.tensor_copy(rhs[:, H : H + 1], ecol[:])
                    nc.tensor.matmul(
                        pool_ps[:], goh_t[:, t * GPC : (t + 1) * GPC], rhs[:],
                        start=(t == 0), stop=(t == NT - 1))

                pooled_raw = smpool.tile([GPC, H + 1], FT, tag="praw")
                nc.vector.tensor_copy(pooled_raw[:], pool_ps[:])
                rec = smpool.tile([GPC, 1], FT, tag="rec")
                nc.vector.reciprocal(rec[:], pooled_raw[:, H : H + 1])
                pooled = smpool.tile([GPC, H], FT, tag="pooled")
                nc.vector.tensor_scalar(
                    out=pooled[:], in0=pooled_raw[:, :H], scalar1=rec[:],
                    scalar2=None, op0=OP.mult)

                def head_mm(x, w, nin, nout, nm, ss=None, badd=None,
                            silu=True):
                    pT = psT.tile([128, 128], FT, tag="tr",
                                  name=f"hT{nm}")
                    nc.tensor.transpose(pT[:nin, :GPC], x[:],
                                        identf_t[:GPC, :GPC])
                    xT = smpool.tile([128, GPC], FT, tag="xT",
                                     name=f"xT{nm}")
                    nc.vector.tensor_copy(xT[:nin, :], pT[:nin, :GPC])
                    ym = psB.tile([128, 256], FT, tag="pB", name=f"ym{nm}")
                    nc.tensor.matmul(ym[:GPC, :nout], xT[:nin, :], w[:],
                                     start=True, stop=True)
                    y = smpool.tile([GPC, nout], FT, tag=f"hd{nout}",
                                    name=f"y{nm}")
                    if ss is not None:
                        nc.vector.tensor_tensor(
                            out=y[:], in0=ym[:GPC, :nout],
                            in1=ss[:GPC, :nout], op=OP.mult)
                        nc.vector.tensor_tensor(
                            out=y[:], in0=y[:], in1=ss[:GPC, nout : 2 * nout],
                            op=OP.add)
                    elif badd is not None:
                        nc.vector.tensor_tensor(
                            out=y[:], in0=ym[:GPC, :nout],
                            in1=badd[:GPC, :nout], op=OP.add)
                    else:
                        nc.vector.tensor_copy(y[:], ym[:GPC, :nout])
                    if silu:
                        ysig = smpool.tile([GPC, nout], FT,
                                           tag=f"hs{nout}", name=f"ys{nm}")
                        nc.scalar.activation(ysig[:], y[:], AF.Sigmoid)
                        nc.vector.tensor_mul(out=y[:], in0=y[:], in1=ysig[:])
                    return y

                y1 = head_mm(pooled, headw1_t, H, H, "a", ss=h1ss_t)
                y2 = head_mm(y1, headw2_t, H, H // 2, "b", ss=h2ss_t)
                y3 = head_mm(y2, headw3_t, H // 2, H // 4, "c", badd=h3b_t)
                y4 = head_mm(y3, headw4_t, H // 4, 1, "d", badd=h4b_t,
                             silu=False)
                nc.sync.dma_start(out=out_d[:], in_=y4[:])

    return nc


_NC_CACHE = None
_LAST_EXEC_NS = None


def kernel(**inputs) -> np.ndarray:
    global _NC_CACHE, _LAST_EXEC_NS
    in_maps = _prep(inputs)
    if _NC_CACHE is None:
        _NC_CACHE = _build()
        _NC_CACHE.finalize()
    trace = os.environ.get("KERNEL_TRACE", "0") == "1"
    res = run_bass_kernel_spmd(
        _NC_CACHE, in_maps, core_ids=list(range(C)), trace=trace
    )
    _LAST_EXEC_NS = res.exec_time_ns
    out = np.concatenate(
        [np.asarray(res.results[c]["out"]).reshape(GPC) for c in range(C)]
    )
    return out.astype(F32)


if __name__ == "__main__":
    import jax

    with jax.default_device(jax.devices("cpu")[0]):
        sys.path.insert(0, os.path.dirname(os.path.abspath(__file__)))
        import reference

        inp = {k: np.asarray(v) for k, v in reference.setup_inputs().items()}
    y = kernel(**inp)
    print("out[:8]:", y[:8])


# revision 11
# speedup vs baseline: 15.7394x; 15.7394x over previous
"""CGCNN regressor on 8 trn2 NeuronCores.

Sharding: graphs 32/core -> contiguous node blocks; edges live on dst's core.
Per core, nodes are permuted into 52 "ranges" of 128 (degree-balanced bin
packing, <=512 edges/range); each range owns 4 edge chunks of 128 slots.
Per layer: h (fp16, scaled by HSC) is AllGathered to a replicated pair-table
[26624, 256]; h[src] is fetched with one dma_gather(transpose=True) per block
(the pair trick keeps indices < 32768 int16) and the even/odd half is merged
in place with copy_predicated. Messages are computed as fp16 matmuls in
natural layout [slots, 256] with everything scaled by HSC so intermediates
fit fp16: src and edge_attr terms via hi/lo fp16 weight pairs, and the dst
term via a precomputed one-hot (sscT) against hi/lo fp16 dst projections.
Per gather block (13 chunks), sigmoid/softplus run as 3 large activations
(sigmoid on both halves, then Ln for a stable softplus), so activation-table
reloads happen twice per block instead of per chunk. Aggregation is a
one-hot matmul into [range,128] PSUM. Pool/head run on 32 graphs/core; host
concatenates the 8x[32] outputs.
"""

import os
import sys

import numpy as np

try:
    import concourse.bass as bass
except ImportError:  # grading env fallback
    sys.path.insert(0, "/opt/trn_rl_repo")
    import concourse.bass as bass

import concourse.mybir as mybir
import concourse.tile as tile
from concourse import bacc
from concourse.bass_utils import run_bass_kernel_spmd

F16 = np.float16
F32 = np.float32

# problem constants
N, E, H, ED, NG, NEMB, L = 50000, 200000, 128, 50, 256, 100, 6
C = 8               # cores
GPC = NG // C       # graphs per core
NT = 52             # node tiles (ranges) per core
N_LOC = NT * 128    # padded local nodes (6656)
CPR = 4             # chunks per range
NCHUNK = NT * CPR   # 208
NSLOT = NCHUNK * 128  # 26624 edge slots
CPB = 13            # chunks per gather block
NBLK = NCHUNK // CPB  # 16
SLOT_B = CPB * 128  # 1664 slots per block
PAIRS = C * N_LOC // 2  # 26624 pair rows in the replicated h table
HSC = 1.0 / 16.0    # fp16 scale: h table, fs, p_fs, msg all carry HSC
SIGC = 6.1e-5       # sigmoid clamp before Ln (fp16 min normal)

_L_RUN = int(os.environ.get("KERNEL_LAYERS", str(L)))
_PHASE = int(os.environ.get("KERNEL_PHASE", "99"))  # 1=proj 2=+conv 99=all
_ABL = set(os.environ.get("KERNEL_ABL", "").split(","))  # timing ablations
_REPS = int(os.environ.get("KERNEL_REPS", "1"))  # repeat body in-NEFF


# ---------------------------------------------------------------------------
# host-side preprocessing
# ---------------------------------------------------------------------------

def _wrap16(idx, pad_to):
    """int16 index tensor in dma_gather layout: [128, pad_to//16],
    slot i -> row i%16, col i//16; replicated 8x down the partitions."""
    a = np.full(pad_to, 0, np.int16)
    a[: len(idx)] = idx.astype(np.int16)
    w = a.reshape(pad_to // 16, 16).T  # [16, pad/16]
    return np.tile(w, (8, 1)).copy()


def _bn_fold(p, bias=None):
    gamma, beta, mean, var = [np.asarray(x, np.float64) for x in p]
    scale = gamma / np.sqrt(var + 1e-5)
    shift = beta - mean * scale
    if bias is not None:
        shift = shift + np.asarray(bias, np.float64) * scale
    return scale.astype(F32), shift.astype(F32)


def _rep(row, parts=128):
    row = np.asarray(row, F32).reshape(1, -1)
    return np.repeat(row, parts, axis=0).copy()


def _hilo(x):
    x = np.asarray(x, F32)
    hi = x.astype(F16)
    lo = (x - hi.astype(F32)).astype(F16)
    return hi, lo


def _prep(inputs):
    x_atom = np.asarray(inputs["x_atom"]).astype(np.int64)
    ei = np.asarray(inputs["edge_index"]).astype(np.int64)
    ea = np.asarray(inputs["edge_attr"]).astype(F32)
    batch = np.asarray(inputs["batch"]).astype(np.int64)
    src, dst = ei[0], ei[1]

    node_start = np.searchsorted(batch, np.arange(0, NG + 1, GPC))
    deg = np.bincount(dst, minlength=N)

    # global node -> (core, local id); degree-balanced FFD into NT ranges/core
    lid = np.empty(N, np.int64)
    core_of = np.empty(N, np.int64)
    for c in range(C):
        s, e = node_start[c], node_start[c + 1]
        nodes = np.arange(s, e)
        assert len(nodes) <= N_LOC, f"core {c}: {len(nodes)} > {N_LOC}"
        order = nodes[np.argsort(-deg[nodes], kind="stable")]
        cap_n = np.full(NT, 128, np.int64)
        cap_e = np.full(NT, CPR * 128, np.int64)
        pos = np.zeros(NT, np.int64)
        for g in order:
            d = deg[g]
            cand = np.where((cap_n > 0) & (cap_e >= d))[0]
            assert len(cand), f"core {c}: range packing failed (deg {d})"
            r = cand[np.argmax(cap_e[cand])]
            lid[g] = r * 128 + pos[r]
            pos[r] += 1
            cap_n[r] -= 1
            cap_e[r] -= d
        core_of[s:e] = c

    gaddr = core_of * N_LOC + lid  # global address in the replicated table

    in_maps = []
    for c in range(C):
        s, e = node_start[c], node_start[c + 1]
        slot_pair = np.zeros(NSLOT, np.int64)
        slot_par = np.zeros(NSLOT, np.uint8)
        slot_dst = np.full(NSLOT, -1, np.int64)  # -1 = pad slot
        slot_ea = np.zeros((NSLOT, ED), F32)
        slot_bias = np.zeros(NSLOT, F32)

        emask = (dst >= s) & (dst < e)
        ce_src, ce_dst, ce_ea = src[emask], dst[emask], ea[emask]
        r_of_e = lid[ce_dst] // 128
        for r in range(NT):
            sel = np.where(r_of_e == r)[0]
            assert len(sel) <= CPR * 128, f"core {c} range {r}: {len(sel)}"
            base = r * CPR * 128
            sl = base + np.arange(len(sel))
            ga = gaddr[ce_src[sel]]
            slot_pair[sl] = ga >> 1
            slot_par[sl] = (ga & 1).astype(np.uint8)
            slot_dst[sl] = lid[ce_dst[sel]] - r * 128
            slot_ea[sl] = ce_ea[sel]
            slot_bias[sl] = 1.0

        # precomputed one-hots: per chunk c, cols [0:128] = ssc (partition =
        # slot, col = dst row), cols [128:256] = sscT (partition = dst row,
        # col = slot)
        ssch = np.zeros((128, NCHUNK, 256), F16)
        sd = slot_dst.reshape(NCHUNK, 128)
        for ch in range(NCHUNK):
            valid = np.where(sd[ch] >= 0)[0]
            dcol = sd[ch][valid]
            ssch[valid, ch, dcol] = 1.0
            ssch[dcol, ch, 128 + valid] = 1.0

        # graph one-hot for pooling over local (permuted) node layout
        goh = np.zeros((128, NT * GPC), F32)
        xa_local = np.zeros(N_LOC, np.int64)
        nodes = np.arange(s, e)
        li = lid[nodes]
        xa_local[li] = x_atom[nodes]
        t_i, p_i = li // 128, li % 128
        goh[p_i, t_i * GPC + (batch[nodes] - c * GPC)] = 1.0

        goh2 = np.zeros((GPC, N_LOC), F32)
        goh2[batch[nodes] - c * GPC, li] = 1.0

        m = {
            "gidx": _wrap16(slot_pair, NSLOT),
            "xidx": _wrap16(xa_local, N_LOC),
            "pmask": np.repeat(
                slot_par.reshape(1, -1), 128, axis=0
            ).astype(np.uint8),
            "eaT": np.concatenate(
                [slot_ea.T, slot_bias.reshape(1, -1)], axis=0
            ).astype(F16),
            "ssch": ssch.reshape(128, NCHUNK * 256),
            "goh": goh,
            "goh2": goh2,
            "maskbias": ((goh - 1.0) * 1e30).astype(F32),
        }
        in_maps.append(m)

    # shared parameters
    conv_Wf = np.asarray(inputs["conv_Wf"], F32)
    conv_Ws = np.asarray(inputs["conv_Ws"], F32)
    conv_bf = np.asarray(inputs["conv_bf"], F32)
    conv_bs = np.asarray(inputs["conv_bs"], F32)
    conv_bn = np.asarray(inputs["conv_bn"], F32)

    # all message terms carry the HSC scale: fs_s = fs * HSC
    # src term: merged (= h*HSC) @ Wsrc  -> weights plain
    wsrc = np.concatenate(
        [
            np.concatenate([conv_Wf[l, H : 2 * H], conv_Ws[l, H : 2 * H]], 1)
            for l in range(L)
        ],
        axis=1,
    )  # [128, L*256]
    # dst term: p_fs = (h @ Wdst) * HSC -> fold HSC into Wdst
    wdst = np.concatenate(
        [
            np.concatenate([conv_Wf[l, :H], conv_Ws[l, :H]], 1)
            for l in range(L)
        ],
        axis=1,
    ) * HSC
    # edge term: ea @ (Wea * HSC) (+ bias * HSC)
    wea = np.concatenate(
        [
            np.concatenate(
                [
                    np.concatenate([conv_Wf[l, 2 * H :], conv_Ws[l, 2 * H :]], 1),
                    np.concatenate([conv_bf[l], conv_bs[l]]).reshape(1, -1),
                ],
                axis=0,
            )
            for l in range(L)
        ],
        axis=1,
    ) * HSC  # [51, L*256]
    wsrc_hi, wsrc_lo = _hilo(wsrc)
    wea_hi, wea_lo = _hilo(wea)
    convss = np.concatenate(
        [
            np.concatenate([_rep(sc), _rep(sh)], axis=1)
            for sc, sh in ((_bn_fold(conv_bn[l])) for l in range(L))
        ],
        axis=1,
    )  # [128, L*256]

    psc, psh = _bn_fold(np.asarray(inputs["proj_bn"], F32),
                        bias=np.asarray(inputs["proj_b"], F32))
    h1sc, h1sh = _bn_fold(np.asarray(inputs["head_bn1"], F32),
                          bias=np.asarray(inputs["head_b1"], F32))
    h2sc, h2sh = _bn_fold(np.asarray(inputs["head_bn2"], F32),
                          bias=np.asarray(inputs["head_b2"], F32))

    shared = {
        "emb": np.asarray(inputs["emb"], F32),
        "projW": np.asarray(inputs["proj_W"], F32),
        "projss": np.concatenate([_rep(psc), _rep(psh)], axis=1),
        "wsrchi": wsrc_hi,
        "wsrclo": wsrc_lo,
        "wdst": wdst.astype(F32),
        "weahi": wea_hi,
        "wealo": wea_lo,
        "convss": convss,
        "gatew1": np.asarray(inputs["gate_W1"], F32),
        "gateb1": _rep(np.asarray(inputs["gate_b1"], F32)),
        "gatew2": np.asarray(inputs["gate_W2"], F32),
        "gateb2": _rep(np.asarray(inputs["gate_b2"], F32).reshape(1)),
        "headw1": np.asarray(inputs["head_W1"], F32),
        "h1ss": np.concatenate([_rep(h1sc), _rep(h1sh)], axis=1),
        "headw2": np.asarray(inputs["head_W2"], F32),
        "h2ss": np.concatenate([_rep(h2sc), _rep(h2sh)], axis=1),
        "headw3": np.asarray(inputs["head_W3"], F32),
        "h3b": _rep(np.asarray(inputs["head_b3"], F32)),
        "headw4": np.asarray(inputs["head_W4"], F32),
        "h4b": _rep(np.asarray(inputs["head_b4"], F32).reshape(1)),
        "identf": np.eye(128, dtype=F32),
    }
    for m in in_maps:
        m.update(shared)
    return in_maps


# ---------------------------------------------------------------------------
# bass program
# ---------------------------------------------------------------------------

def _build():
    dt = mybir.dt
    nc = bacc.Bacc(num_devices=C)

    def par(name, shape, dtp):
        return nc.declare_dram_parameter(name, list(shape), dtp, isOutput=False)

    gidx_d = par("gidx", [128, NSLOT // 16], dt.int16)
    xidx_d = par("xidx", [128, N_LOC // 16], dt.int16)
    pmask_d = par("pmask", [128, NSLOT], dt.uint8)
    eaT_d = par("eaT", [ED + 1, NSLOT], dt.float16)
    ssch_d = par("ssch", [128, NCHUNK * 256], dt.float16)
    goh_d = par("goh", [128, NT * GPC], dt.float32)
    goh2_d = par("goh2", [GPC, N_LOC], dt.float32)
    maskbias_d = par("maskbias", [128, NT * GPC], dt.float32)
    emb_d = par("emb", [NEMB, H], dt.float32)
    projW_d = par("projW", [H, H], dt.float32)
    projss_d = par("projss", [128, 256], dt.float32)
    wsrchi_d = par("wsrchi", [H, L * 256], dt.float16)
    wsrclo_d = par("wsrclo", [H, L * 256], dt.float16)
    wdst_d = par("wdst", [H, L * 256], dt.float32)
    weahi_d = par("weahi", [ED + 1, L * 256], dt.float16)
    wealo_d = par("wealo", [ED + 1, L * 256], dt.float16)
    convss_d = par("convss", [128, L * 256], dt.float32)
    gatew1_d = par("gatew1", [H, H // 2], dt.float32)
    gateb1_d = par("gateb1", [128, H // 2], dt.float32)
    gatew2_d = par("gatew2", [H // 2, 1], dt.float32)
    gateb2_d = par("gateb2", [128, 1], dt.float32)
    headw1_d = par("headw1", [H, H], dt.float32)
    h1ss_d = par("h1ss", [128, 256], dt.float32)
    headw2_d = par("headw2", [H, H // 2], dt.float32)
    h2ss_d = par("h2ss", [128, 128], dt.float32)
    headw3_d = par("headw3", [H // 2, H // 4], dt.float32)
    h3b_d = par("h3b", [128, H // 4], dt.float32)
    headw4_d = par("headw4", [H // 4, 1], dt.float32)
    h4b_d = par("h4b", [128, 1], dt.float32)
    identf_d = par("identf", [128, 128], dt.float32)

    out_d = nc.declare_dram_parameter("out", [GPC, 1], dt.float32,
                                      isOutput=True)

    hstage = nc.dram_tensor("hstage", [N_LOC // 2, 256], dt.float16)
    hfull = [
        nc.dram_tensor(f"hfull{i}", [PAIRS, 256], dt.float16,
                       addr_space="Shared")
        for i in range(2)
    ]

    FT, F16T = dt.float32, dt.float16
    AF = mybir.ActivationFunctionType
    OP = mybir.AluOpType

    with tile.TileContext(nc) as tc:
        with (
            tc.tile_pool(name="const", bufs=1) as cpool,
            tc.tile_pool(name="state", bufs=1) as spool,
            tc.tile_pool(name="psA", bufs=2, space="PSUM") as psA,
            tc.tile_pool(name="psB", bufs=2, space="PSUM") as psB,
            tc.tile_pool(name="psT", bufs=2, space="PSUM") as psT,
            tc.tile_pool(name="psG", bufs=2, space="PSUM") as psG,
        ):
            # ---------------- resident tiles ----------------
            def load(pool, dram, shape, dtp, sfx=""):
                nm = f"c_{dram.name}{sfx}"
                t = pool.tile(shape, dtp, name=nm, tag=nm)
                nc.sync.dma_start(out=t[:], in_=dram[:])
                return t

            gidx_t = load(cpool, gidx_d, [128, NSLOT // 16], dt.int16)
            projW_t = load(cpool, projW_d, [H, H], FT)
            projss_t = load(cpool, projss_d, [128, 256], FT)
            wsrchi_t = load(cpool, wsrchi_d, [H, L * 256], F16T)
            wsrclo_t = load(cpool, wsrclo_d, [H, L * 256], F16T)
            wdst_t = load(cpool, wdst_d, [H, L * 256], FT)
            weahi_t = load(cpool, weahi_d, [ED + 1, L * 256], F16T)
            wealo_t = load(cpool, wealo_d, [ED + 1, L * 256], F16T)
            convss_t = load(cpool, convss_d, [128, L * 256], FT)
            identf_t = load(cpool, identf_d, [128, 128], FT)

            h_loc = spool.tile([128, NT, H], FT, tag="h_loc")
            pfs_hi = spool.tile([128, NT, 256], F16T, tag="pfs_hi")
            pfs_lo = spool.tile([128, NT, 256], F16T, tag="pfs_lo")
            h_bf = spool.tile([128, NT, H], F16T, tag="h_bf")

            for rep in range(_REPS):
                R = f"R{rep}_"

                # ---------------- embedding + projection ----------------
                with (
                    tc.tile_pool(name=f"{R}proj", bufs=2) as prpool,
                    tc.tile_pool(name=f"{R}projc", bufs=1) as prcpool,
                ):
                    xidx_t = load(prcpool, xidx_d, [128, N_LOC // 16],
                                  dt.int16, sfx=R)
                    TPG = 13  # node tiles per gather call
                    for g in range(NT // TPG):
                        h0 = prpool.tile([128, TPG, H], FT, tag="h0")
                        nc.gpsimd.dma_gather(
                            h0[:], emb_d[:],
                            xidx_t[:, g * (TPG * 8) : (g + 1) * (TPG * 8)],
                            TPG * 128, TPG * 128, H, single_packet=False,
                        )
                        for tt in range(TPG):
                            t = g * TPG + tt
                            pT = psT.tile([128, 128], FT, tag="tr",
                                          name=f"{R}prT{t}")
                            nc.tensor.transpose(pT[:], h0[:, tt, :],
                                                identf_t[:])
                            hT = prpool.tile([128, 128], FT, tag="hT32",
                                             name=f"{R}prh{t}")
                            nc.vector.tensor_copy(hT[:], pT[:])
                            pm = psB.tile([128, 256], FT, tag="pB",
                                          name=f"{R}prm{t}")
                            nc.tensor.matmul(pm[:, :H], hT[:], projW_t[:],
                                             start=True, stop=True)
                            t1 = prpool.tile([128, H], FT, tag="nupd",
                                             name=f"{R}pru{t}")
                            nc.vector.tensor_tensor(
                                out=t1[:], in0=pm[:, :H],
                                in1=projss_t[:, :128], op=OP.mult)
                            nc.vector.tensor_tensor(
                                out=t1[:], in0=t1[:], in1=projss_t[:, 128:],
                                op=OP.add)
                            sgp = prpool.tile([128, H], FT, tag="sgp",
                                              name=f"{R}prs{t}")
                            nc.scalar.activation(sgp[:], t1[:], AF.Sigmoid)
                            nc.vector.tensor_mul(out=h_loc[:, t, :],
                                                 in0=t1[:], in1=sgp[:])

                if _PHASE <= 1:
                    dbg = spool.tile([GPC, 1], FT, tag="dbg",
                                     name=f"{R}dbg1")
                    nc.vector.tensor_copy(dbg[:], h_loc[:GPC, 0, 0:1])
                    nc.sync.dma_start(out=out_d[:], in_=dbg[:])

                # ---------------- conv layers ----------------
                with (
                    tc.tile_pool(name=f"{R}gbuf", bufs=2) as gpool,
                    tc.tile_pool(name=f"{R}work", bufs=2) as wpool,
                    tc.tile_pool(name=f"{R}wk1", bufs=1) as w1pool,
                    tc.tile_pool(name=f"{R}small", bufs=3) as smpool,
                ):
                    for l in range(_L_RUN if _PHASE >= 2 else 0):
                        hf = hfull[l % 2]
                        # stage h as scaled fp16 + allgather
                        nc.vector.tensor_scalar_mul(
                            out=h_bf[:].rearrange("p t h -> p (t h)"),
                            in0=h_loc[:].rearrange("p t h -> p (t h)"),
                            scalar1=HSC)
                        nc.sync.dma_start(
                            out=hstage[:]
                            .rearrange("n (two h) -> (n two) h", two=2)
                            .rearrange("(t p) h -> p t h", p=128),
                            in_=h_bf[:],
                        )
                        nc.gpsimd.collective_compute(
                            "AllGather",
                            mybir.AluOpType.bypass,
                            replica_groups=[list(range(C))],
                            ins=[hstage[:]],
                            outs=[hf[:]],
                        )

                        # dst-side node projections pfs = (h@Wdst)*HSC, hi/lo
                        for t in range(NT):
                            pT = psT.tile([128, 128], FT, tag="tr",
                                          name=f"{R}pT_{l}_{t}")
                            nc.tensor.transpose(pT[:], h_loc[:, t, :],
                                                identf_t[:])
                            hTb = wpool.tile([128, 128], FT, tag="hTb",
                                             name=f"{R}hTb_{l}_{t}")
                            nc.vector.tensor_copy(hTb[:], pT[:])
                            pm = psB.tile([128, 256], FT, tag="pB",
                                          name=f"{R}pm_{l}_{t}")
                            nc.tensor.matmul(
                                pm[:], hTb[:],
                                wdst_t[:, l * 256 : (l + 1) * 256],
                                start=True, stop=True)
                            nc.vector.tensor_copy(pfs_hi[:, t, :], pm[:])
                            nc.vector.tensor_tensor(
                                out=pfs_lo[:, t, :], in0=pm[:],
                                in1=pfs_hi[:, t, :], op=OP.subtract)

                        aggr = {}
                        for b in range(NBLK):
                            bsl = slice(b * SLOT_B, (b + 1) * SLOT_B)
                            gb = gpool.tile([128, 2, SLOT_B], F16T, tag="gb",
                                            name=f"{R}gb_{l}_{b}")
                            if "nog" not in _ABL:
                                nc.gpsimd.dma_gather(
                                    gb[:], hf[:],
                                    gidx_t[:, b * (SLOT_B // 16)
                                           : (b + 1) * (SLOT_B // 16)],
                                    SLOT_B, SLOT_B, 256, transpose=True,
                                    single_packet=False,
                                )
                            mask = wpool.tile([128, SLOT_B], dt.uint8,
                                              tag="mask", name=f"{R}mk_{l}_{b}")
                            if "noea" not in _ABL:
                                nc.sync.dma_start(out=mask[:],
                                                  in_=pmask_d[:, bsl])
                            # merge even/odd half in place
                            if "nopred" not in _ABL:
                                nc.vector.copy_predicated(gb[:, 0, :],
                                                          mask[:],
                                                          gb[:, 1, :])
                            ea_t = wpool.tile([ED + 1, SLOT_B], F16T,
                                              tag="ea", name=f"{R}ea_{l}_{b}")
                            if "noea" not in _ABL:
                                nc.sync.dma_start(out=ea_t[:],
                                                  in_=eaT_d[:, bsl])
                            ssk = wpool.tile([128, CPB, 256], F16T,
                                             tag="ssk", name=f"{R}ssk_{l}_{b}")
                            if "nossk" not in _ABL:
                                nc.sync.dma_start(
                                    out=ssk[:],
                                    in_=ssch_d[:, b * CPB * 256
                                               : (b + 1) * CPB * 256])

                            # fs_s = fs*HSC per chunk; stashed fp16 as
                            # [128, 2(half), CPB, 128]
                            fsacc = wpool.tile([128, 2, CPB, 128], F16T,
                                               tag="fsacc",
                                               name=f"{R}fsa_{l}_{b}")
                            for j in range(CPB):
                                c = b * CPB + j
                                r = c // CPR
                                sl = slice(j * 128, (j + 1) * 128)
                                fs = psA.tile([128, 256], FT, tag="fs",
                                              name=f"{R}fs_{l}_{c}")
                                lsl = slice(l * 256, (l + 1) * 256)
                                if "nomm" in _ABL:
                                    continue
                                nc.tensor.matmul(fs[:], gb[:, 0, sl],
                                                 wsrchi_t[:, lsl],
                                                 start=True, stop=False)
                                nc.tensor.matmul(fs[:], gb[:, 0, sl],
                                                 wsrclo_t[:, lsl],
                                                 start=False, stop=False)
                                nc.tensor.matmul(fs[:], ea_t[:, sl],
                                                 weahi_t[:, lsl],
                                                 start=False, stop=False)
                                nc.tensor.matmul(fs[:], ea_t[:, sl],
                                                 wealo_t[:, lsl],
                                                 start=False, stop=False)
                                nc.tensor.matmul(fs[:], ssk[:, j, 128:256],
                                                 pfs_hi[:, r, :],
                                                 start=False, stop=False)
                                nc.tensor.matmul(fs[:], ssk[:, j, 128:256],
                                                 pfs_lo[:, r, :],
                                                 start=False, stop=True)
                                if "nocp" not in _ABL:
                                    nc.vector.tensor_copy(
                                        fsacc[:, :, j, :],
                                        fs[:].rearrange(
                                            "p (two h) -> p two h", two=2))

                            # block activations: sigmoid(f), sigmoid(-s), Ln
                            if "noact" in _ABL:
                                continue
                            sgf = w1pool.tile([128, CPB, 128], F16T,
                                              tag="sgf", name=f"{R}sgf_{l}_{b}")
                            nc.scalar.activation(
                                sgf[:].rearrange("p c h -> p (c h)"),
                                fsacc[:, 0, :, :]
                                .rearrange("p c h -> p (c h)"),
                                AF.Sigmoid, scale=1.0 / HSC)
                            sgc = wpool.tile([128, CPB, 128], F16T,
                                             tag="sgc", name=f"{R}sgc_{l}_{b}")
                            nc.scalar.activation(
                                sgc[:].rearrange("p c h -> p (c h)"),
                                fsacc[:, 1, :, :]
                                .rearrange("p c h -> p (c h)"),
                                AF.Sigmoid, scale=-1.0 / HSC)
                            spc = w1pool.tile([128, CPB, 128], F16T,
                                              tag="spc", name=f"{R}spc_{l}_{b}")
                            nc.vector.tensor_scalar_max(
                                out=spc[:].rearrange("p c h -> p (c h)"),
                                in0=sgc[:].rearrange("p c h -> p (c h)"),
                                scalar1=SIGC)
                            lnv = w1pool.tile([128, CPB, 128], FT,
                                              tag="lnv", name=f"{R}lnv_{l}_{b}")
                            nc.scalar.activation(
                                lnv[:].rearrange("p c h -> p (c h)"),
                                spc[:].rearrange("p c h -> p (c h)"), AF.Ln)
                            # sp_s = max(-ln(sigc)*HSC, s_s)
                            spv = wpool.tile([128, CPB, 128], F16T,
                                             tag="spv", name=f"{R}spv_{l}_{b}")
                            nc.vector.scalar_tensor_tensor(
                                out=spv[:].rearrange("p c h -> p (c h)"),
                                in0=lnv[:].rearrange("p c h -> p (c h)"),
                                scalar=-HSC,
                                in1=fsacc[:, 1, :, :]
                                .rearrange("p c h -> p (c h)"),
                                op0=OP.mult, op1=OP.max)
                            msgb = wpool.tile([128, CPB, 128], F16T,
                                              tag="msgb", name=f"{R}msg_{l}_{b}")
                            nc.vector.tensor_mul(
                                out=msgb[:].rearrange("p c h -> p (c h)"),
                                in0=sgf[:].rearrange("p c h -> p (c h)"),
                                in1=spv[:].rearrange("p c h -> p (c h)"))

                            for j in range(CPB):
                                c = b * CPB + j
                                r = c // CPR
                                if c % CPR == 0:
                                    aggr[r] = psG.tile([128, H], FT,
                                                       tag="aggr",
                                                       name=f"{R}aggr_{l}_{r}")
                                nc.tensor.matmul(
                                    aggr[r][:], ssk[:, j, 0:128],
                                    msgb[:, j, :],
                                    start=(c % CPR == 0),
                                    stop=(c % CPR == CPR - 1))
                                if c % CPR == CPR - 1:
                                    lss = convss_t[:, l * 256 : (l + 1) * 256]
                                    u = smpool.tile([128, H], FT, tag="nupd",
                                                    name=f"{R}u_{l}_{r}")
                                    nc.vector.scalar_tensor_tensor(
                                        out=u[:], in0=aggr[r][:],
                                        scalar=1.0 / HSC,
                                        in1=h_loc[:, r, :],
                                        op0=OP.mult, op1=OP.add)
                                    nc.vector.tensor_tensor(
                                        out=u[:], in0=u[:], in1=lss[:, :128],
                                        op=OP.mult)
                                    nc.vector.tensor_tensor(
                                        out=u[:], in0=u[:], in1=lss[:, 128:],
                                        op=OP.add)
                                    us = smpool.tile([128, H], FT,
                                                     tag="nsig",
                                                     name=f"{R}us_{l}_{r}")
                                    nc.scalar.activation(us[:], u[:],
                                                         AF.Sigmoid)
                                    nc.vector.tensor_mul(out=us[:], in0=u[:],
                                                         in1=us[:])
                                    nc.vector.tensor_tensor(
                                        out=h_loc[:, r, :], in0=us[:],
                                        in1=h_loc[:, r, :], op=OP.add)
                                    del aggr[r]

                if _PHASE == 2:
                    dbg2 = spool.tile([GPC, 1], FT, tag="dbg",
                                      name=f"{R}dbg2")
                    nc.vector.tensor_copy(dbg2[:], h_loc[:GPC, 0, 0:1])
                    nc.sync.dma_start(out=out_d[:], in_=dbg2[:])

                # ---------------- gate + pooling + head ----------------
                with (
                    tc.tile_pool(name=f"{R}poolc", bufs=1) as pcpool,
                    tc.tile_pool(name=f"{R}pools", bufs=3) as smpool,
                ):
                  if _PHASE >= 5:
                    goh_t = load(pcpool, goh_d, [128, NT * GPC], FT, sfx=R)
                    goh2_t = load(pcpool, goh2_d, [GPC, N_LOC], FT, sfx=R)
                    maskb_t = load(pcpool, maskbias_d, [128, NT * GPC], FT,
                                   sfx=R)
                    gatew1_t = load(pcpool, gatew1_d, [H, H // 2], FT, sfx=R)
                    gateb1_t = load(pcpool, gateb1_d, [128, H // 2], FT,
                                    sfx=R)
                    gatew2_t = load(pcpool, gatew2_d, [H // 2, 1], FT, sfx=R)
                    gateb2_t = load(pcpool, gateb2_d, [128, 1], FT, sfx=R)
                    headw1_t = load(pcpool, headw1_d, [H, H], FT, sfx=R)
                    h1ss_t = load(pcpool, h1ss_d, [128, 256], FT, sfx=R)
                    headw2_t = load(pcpool, headw2_d, [H, H // 2], FT, sfx=R)
                    h2ss_t = load(pcpool, h2ss_d, [128, 128], FT, sfx=R)
                    headw3_t = load(pcpool, headw3_d, [H // 2, H // 4], FT,
                                    sfx=R)
                    h3b_t = load(pcpool, h3b_d, [128, H // 4], FT, sfx=R)
                    headw4_t = load(pcpool, headw4_d, [H // 4, 1], FT, sfx=R)
                    h4b_t = load(pcpool, h4b_d, [128, 1], FT, sfx=R)

                    g_all = pcpool.tile([128, NT], FT, name=f"{R}g_all",
                                        tag="g_all")
                    runmax = pcpool.tile([128, GPC], FT, name=f"{R}runmax",
                                         tag="runmax")

                    # pass 1: per-node gate scores g + running per-graph max
                    for t in range(NT):
                        pT = psT.tile([128, 128], FT, tag="tr",
                                      name=f"{R}gT{t}")
                        nc.tensor.transpose(pT[:], h_loc[:, t, :],
                                            identf_t[:])
                        hT = smpool.tile([128, 128], FT, tag="hT32",
                                         name=f"{R}gh{t}")
                        nc.vector.tensor_copy(hT[:], pT[:])
                        g1 = psB.tile([128, 256], FT, tag="pB",
                                      name=f"{R}g1_{t}")
                        nc.tensor.matmul(g1[:, : H // 2], hT[:], gatew1_t[:],
                                         start=True, stop=True)
                        s1 = smpool.tile([128, H // 2], FT, tag="s1",
                                         name=f"{R}s1_{t}")
                        nc.vector.tensor_tensor(
                            out=s1[:], in0=g1[:, : H // 2], in1=gateb1_t[:],
                            op=OP.add)
                        s1s = smpool.tile([128, H // 2], FT, tag="s1s",
                                          name=f"{R}s1s_{t}")
                        nc.scalar.activation(s1s[:], s1[:], AF.Sigmoid)
                        nc.vector.tensor_mul(out=s1[:], in0=s1[:],
                                             in1=s1s[:])
                        pT2 = psT.tile([128, 128], FT, tag="tr",
                                       name=f"{R}gU{t}")
                        nc.tensor.transpose(pT2[: H // 2, :], s1[:],
                                            identf_t[:])
                        s1T = smpool.tile([H // 2, 128], FT, tag="s1T",
                                          name=f"{R}s1T_{t}")
                        nc.vector.tensor_copy(s1T[:], pT2[: H // 2, :])
                        g2 = psT.tile([128, 128], FT, tag="tr",
                                      name=f"{R}g2_{t}")
                        nc.tensor.matmul(g2[:, :1], s1T[:], gatew2_t[:],
                                         start=True, stop=True)
                        nc.vector.tensor_tensor(
                            out=g_all[:, t : t + 1], in0=g2[:, :1],
                            in1=gateb2_t[:], op=OP.add)
                        gm = smpool.tile([128, GPC], FT, tag="gm",
                                         name=f"{R}gm_{t}")
                        nc.vector.tensor_tensor(
                            out=gm[:],
                            in0=g_all[:, t : t + 1].to_broadcast([128, GPC]),
                            in1=goh_t[:, t * GPC : (t + 1) * GPC],
                            op=OP.mult)
                        nc.vector.tensor_tensor(
                            out=gm[:], in0=gm[:],
                            in1=maskb_t[:, t * GPC : (t + 1) * GPC],
                            op=OP.add)
                        if t == 0:
                            nc.vector.tensor_copy(runmax[:], gm[:])
                        else:
                            nc.vector.tensor_max(out=runmax[:],
                                                 in0=runmax[:], in1=gm[:])

                    # reduce running max across partitions -> gmax [GPC, 1]
                    pTm = psT.tile([128, 128], FT, tag="tr", name=f"{R}pTm")
                    nc.tensor.transpose(pTm[:GPC, :], runmax[:], identf_t[:])
                    rmT = smpool.tile([GPC, 128], FT, tag="rmT",
                                      name=f"{R}rmT")
                    nc.vector.tensor_copy(rmT[:], pTm[:GPC, :])
                    negmax = smpool.tile([GPC, 1], FT, tag="negmax",
                                         name=f"{R}negmax")
                    nc.vector.tensor_reduce(out=negmax[:], in_=rmT[:],
                                            axis=mybir.AxisListType.X,
                                            op=OP.max)
                    nc.vector.tensor_scalar_mul(out=negmax[:], in0=negmax[:],
                                                scalar1=-1.0)

                    # pass 2: e = exp(min(g - gmax[graph], 20)), pooled sums
                    pool_ps = psA.tile([GPC, H + 1], FT, tag="fs",
                                       name=f"{R}pool_ps")
                    for t in range(NT):
                        nK = psT.tile([128, 128], FT, tag="tr",
                                      name=f"{R}nK{t}")
                        nc.tensor.matmul(
                            nK[:, :1], goh2_t[:, t * 128 : (t + 1) * 128],
                            negmax[:], start=True, stop=True)
                        earg = smpool.tile([128, 1], FT, tag="earg",
                                           name=f"{R}ea2_{t}")
                        nc.vector.tensor_tensor(
                            out=earg[:], in0=g_all[:, t : t + 1],
                            in1=nK[:, :1], op=OP.add)
                        nc.vector.tensor_scalar_min(out=earg[:], in0=earg[:],
                                                    scalar1=20.0)
                        ecol = smpool.tile([128, 1], FT, tag="ecol",
                                           name=f"{R}ec_{t}")
                        nc.scalar.activation(ecol[:], earg[:], AF.Exp)
                        rhs = smpool.tile([128, H + 1], FT, tag="rhs",
                                          name=f"{R}rhs_{t}")
                        nc.vector.tensor_scalar(
                            out=rhs[:, :H], in0=h_loc[:, t, :],
                            scalar1=ecol[:], scalar2=None, op0=OP.mult)
                        nc.vector.tensor_copy(rhs[:, H : H + 1], ecol[:])
                        nc.tensor.matmul(
                            pool_ps[:], goh_t[:, t * GPC : (t + 1) * GPC],
                            rhs[:], start=(t == 0), stop=(t == NT - 1))

                    pooled_raw = smpool.tile([GPC, H + 1], FT, tag="praw",
                                             name=f"{R}praw")
                    nc.vector.tensor_copy(pooled_raw[:], pool_ps[:])
                    rec = smpool.tile([GPC, 1], FT, tag="rec", name=f"{R}rec")
                    nc.vector.reciprocal(rec[:], pooled_raw[:, H : H + 1])
                    pooled = smpool.tile([GPC, H], FT, tag="pooled",
                                         name=f"{R}pooled")
                    nc.vector.tensor_scalar(
                        out=pooled[:], in0=pooled_raw[:, :H], scalar1=rec[:],
                        scalar2=None, op0=OP.mult)

                    def head_mm(x, w, nin, nout, nm, ss=None, badd=None,
                                silu=True):
                        pT = psT.tile([128, 128], FT, tag="tr",
                                      name=f"{R}hT{nm}")
                        nc.tensor.transpose(pT[:nin, :GPC], x[:],
                                            identf_t[:GPC, :GPC])
                        xT = smpool.tile([128, GPC], FT, tag="xT",
                                         name=f"{R}xT{nm}")
                        nc.vector.tensor_copy(xT[:nin, :], pT[:nin, :GPC])
                        ym = psB.tile([128, 256], FT, tag="pB",
                                      name=f"{R}ym{nm}")
                        nc.tensor.matmul(ym[:GPC, :nout], xT[:nin, :], w[:],
                                         start=True, stop=True)
                        y = smpool.tile([GPC, nout], FT, tag=f"hd{nout}",
                                        name=f"{R}y{nm}")
                        if ss is not None:
                            nc.vector.tensor_tensor(
                                out=y[:], in0=ym[:GPC, :nout],
                                in1=ss[:GPC, :nout], op=OP.mult)
                            nc.vector.tensor_tensor(
                                out=y[:], in0=y[:],
                                in1=ss[:GPC, nout : 2 * nout], op=OP.add)
                        elif badd is not None:
                            nc.vector.tensor_tensor(
                                out=y[:], in0=ym[:GPC, :nout],
                                in1=badd[:GPC, :nout], op=OP.add)
                        else:
                            nc.vector.tensor_copy(y[:], ym[:GPC, :nout])
                        if silu:
                            ysig = smpool.tile([GPC, nout], FT,
                                               tag=f"hs{nout}",
                                               name=f"{R}ys{nm}")
                            nc.scalar.activation(ysig[:], y[:], AF.Sigmoid)
                            nc.vector.tensor_mul(out=y[:], in0=y[:],
                                                 in1=ysig[:])
                        return y

                    y1 = head_mm(pooled, headw1_t, H, H, "a", ss=h1ss_t)
                    y2 = head_mm(y1, headw2_t, H, H // 2, "b", ss=h2ss_t)
                    y3 = head_mm(y2, headw3_t, H // 2, H // 4, "c",
                                 badd=h3b_t)
                    y4 = head_mm(y3, headw4_t, H // 4, 1, "d", badd=h4b_t,
                                 silu=False)
                    nc.sync.dma_start(out=out_d[:], in_=y4[:])

    return nc


_NC_CACHE = None
_LAST_EXEC_NS = None


def kernel(**inputs) -> np.ndarray:
    global _NC_CACHE, _LAST_EXEC_NS
    in_maps = _prep(inputs)
    if _NC_CACHE is None:
        _NC_CACHE = _build()
        _NC_CACHE.finalize()
    trace = os.environ.get("KERNEL_TRACE", "0") == "1"
    res = run_bass_kernel_spmd(
        _NC_CACHE, in_maps, core_ids=list(range(C)), trace=trace
    )
    _LAST_EXEC_NS = res.exec_time_ns
    out = np.concatenate(
        [np.asarray(res.results[c]["out"]).reshape(GPC) for c in range(C)]
    )
    return out.astype(F32)


if __name__ == "__main__":
    import jax

    with jax.default_device(jax.devices("cpu")[0]):
        sys.path.insert(0, os.path.dirname(os.path.abspath(__file__)))
        import reference

        inp = {k: np.asarray(v) for k, v in reference.setup_inputs().items()}
    y = kernel(**inp)
    print("out[:8]:", y[:8])


# revision 14
# speedup vs baseline: 16.1180x; 1.0241x over previous
"""CGCNN regressor on 8 trn2 NeuronCores.

Sharding: graphs 32/core -> contiguous node blocks; edges live on dst's core.
Per core, nodes are permuted into 52 "ranges" of 128 (degree-balanced bin
packing, <=512 edges/range); each range owns 4 edge chunks of 128 slots.
Per layer: h (fp16, scaled by HSC) is AllGathered to a replicated pair-table
[26624, 256]; h[src] is fetched with one dma_gather(transpose=True) per block
(the pair trick keeps indices < 32768 int16) and the even/odd half is merged
in place with copy_predicated. Messages are computed as fp16 matmuls in
natural layout [slots, 256] with everything scaled by HSC so intermediates
fit fp16: src and edge_attr terms via hi/lo fp16 weight pairs, and the dst
term via a precomputed one-hot (sscT) against hi/lo fp16 dst projections.
Per gather block (13 chunks), sigmoid/softplus run as 3 large activations
(sigmoid on both halves, then Ln for a stable softplus), so activation-table
reloads happen twice per block instead of per chunk. Aggregation is a
one-hot matmul into [range,128] PSUM. Pool/head run on 32 graphs/core; host
concatenates the 8x[32] outputs.
"""

import os
import sys

import numpy as np

try:
    import concourse.bass as bass
except ImportError:  # grading env fallback
    sys.path.insert(0, "/opt/trn_rl_repo")
    import concourse.bass as bass

import concourse.mybir as mybir
import concourse.tile as tile
from concourse import bacc
from concourse.bass_utils import run_bass_kernel_spmd

F16 = np.float16
F32 = np.float32

# problem constants
N, E, H, ED, NG, NEMB, L = 50000, 200000, 128, 50, 256, 100, 6
C = 8               # cores
GPC = NG // C       # graphs per core
NT = 52             # node tiles (ranges) per core
N_LOC = NT * 128    # padded local nodes (6656)
CPR = 4             # chunks per range
NCHUNK = NT * CPR   # 208
NSLOT = NCHUNK * 128  # 26624 edge slots
CPB = 13            # chunks per gather block
NBLK = NCHUNK // CPB  # 16
SLOT_B = CPB * 128  # 1664 slots per block
PAIRS = C * N_LOC // 2  # 26624 pair rows in the replicated h table
GSPLIT = 2          # AllGather split: groups of NT/GSPLIT ranges
ROWS_G = N_LOC // 2 // GSPLIT  # pair rows per (core, group)
HSC = 1.0 / 16.0    # fp16 scale: h table, fs, p_fs, msg all carry HSC
SIGC = 6.1e-5       # sigmoid clamp before Ln (fp16 min normal)

_L_RUN = int(os.environ.get("KERNEL_LAYERS", str(L)))
_PHASE = int(os.environ.get("KERNEL_PHASE", "99"))  # 1=proj 2=+conv 99=all
_ABL = set(os.environ.get("KERNEL_ABL", "").split(","))  # timing ablations
_REPS = int(os.environ.get("KERNEL_REPS", "1"))  # repeat body in-NEFF


# ---------------------------------------------------------------------------
# host-side preprocessing
# ---------------------------------------------------------------------------

def _wrap16(idx, pad_to):
    """int16 index tensor in dma_gather layout: [128, pad_to//16],
    slot i -> row i%16, col i//16; replicated 8x down the partitions."""
    a = np.full(pad_to, 0, np.int16)
    a[: len(idx)] = idx.astype(np.int16)
    w = a.reshape(pad_to // 16, 16).T  # [16, pad/16]
    return np.tile(w, (8, 1)).copy()


def _bn_fold(p, bias=None):
    gamma, beta, mean, var = [np.asarray(x, np.float64) for x in p]
    scale = gamma / np.sqrt(var + 1e-5)
    shift = beta - mean * scale
    if bias is not None:
        shift = shift + np.asarray(bias, np.float64) * scale
    return scale.astype(F32), shift.astype(F32)


def _rep(row, parts=128):
    row = np.asarray(row, F32).reshape(1, -1)
    return np.repeat(row, parts, axis=0).copy()


def _hilo(x):
    x = np.asarray(x, F32)
    hi = x.astype(F16)
    lo = (x - hi.astype(F32)).astype(F16)
    return hi, lo


def _prep(inputs):
    x_atom = np.asarray(inputs["x_atom"]).astype(np.int64)
    ei = np.asarray(inputs["edge_index"]).astype(np.int64)
    ea = np.asarray(inputs["edge_attr"]).astype(F32)
    batch = np.asarray(inputs["batch"]).astype(np.int64)
    src, dst = ei[0], ei[1]

    node_start = np.searchsorted(batch, np.arange(0, NG + 1, GPC))
    deg = np.bincount(dst, minlength=N)

    # global node -> (core, local id); degree-balanced FFD into NT ranges/core
    lid = np.empty(N, np.int64)
    core_of = np.empty(N, np.int64)
    for c in range(C):
        s, e = node_start[c], node_start[c + 1]
        nodes = np.arange(s, e)
        assert len(nodes) <= N_LOC, f"core {c}: {len(nodes)} > {N_LOC}"
        order = nodes[np.argsort(-deg[nodes], kind="stable")]
        cap_n = np.full(NT, 128, np.int64)
        cap_e = np.full(NT, CPR * 128, np.int64)
        pos = np.zeros(NT, np.int64)
        for g in order:
            d = deg[g]
            cand = np.where((cap_n > 0) & (cap_e >= d))[0]
            assert len(cand), f"core {c}: range packing failed (deg {d})"
            r = cand[np.argmax(cap_e[cand])]
            lid[g] = r * 128 + pos[r]
            pos[r] += 1
            cap_n[r] -= 1
            cap_e[r] -= d
        core_of[s:e] = c

    # pair-row address in the group-major replicated table:
    # row = g*(C*ROWS_G) + core*ROWS_G + (p_local % ROWS_G), parity = lid&1
    p_local = lid >> 1
    grp = p_local // ROWS_G
    pair_row = grp * (C * ROWS_G) + core_of * ROWS_G + (p_local % ROWS_G)

    in_maps = []
    for c in range(C):
        s, e = node_start[c], node_start[c + 1]
        slot_pair = np.zeros(NSLOT, np.int64)
        slot_par = np.zeros(NSLOT, np.uint8)
        slot_dst = np.full(NSLOT, -1, np.int64)  # -1 = pad slot
        slot_ea = np.zeros((NSLOT, ED), F32)
        slot_bias = np.zeros(NSLOT, F32)

        emask = (dst >= s) & (dst < e)
        ce_src, ce_dst, ce_ea = src[emask], dst[emask], ea[emask]
        r_of_e = lid[ce_dst] // 128
        for r in range(NT):
            sel = np.where(r_of_e == r)[0]
            assert len(sel) <= CPR * 128, f"core {c} range {r}: {len(sel)}"
            base = r * CPR * 128
            sl = base + np.arange(len(sel))
            slot_pair[sl] = pair_row[ce_src[sel]]
            slot_par[sl] = (lid[ce_src[sel]] & 1).astype(np.uint8)
            slot_dst[sl] = lid[ce_dst[sel]] - r * 128
            slot_ea[sl] = ce_ea[sel]
            slot_bias[sl] = 1.0

        # precomputed one-hots: per chunk c, cols [0:128] = ssc (partition =
        # slot, col = dst row), cols [128:256] = sscT (partition = dst row,
        # col = slot)
        ssch = np.zeros((128, NCHUNK, 256), F16)
        sd = slot_dst.reshape(NCHUNK, 128)
        for ch in range(NCHUNK):
            valid = np.where(sd[ch] >= 0)[0]
            dcol = sd[ch][valid]
            ssch[valid, ch, dcol] = 1.0
            ssch[dcol, ch, 128 + valid] = 1.0

        # graph one-hot for pooling over local (permuted) node layout
        goh = np.zeros((128, NT * GPC), F32)
        xa_local = np.zeros(N_LOC, np.int64)
        nodes = np.arange(s, e)
        li = lid[nodes]
        xa_local[li] = x_atom[nodes]
        t_i, p_i = li // 128, li % 128
        goh[p_i, t_i * GPC + (batch[nodes] - c * GPC)] = 1.0

        goh2 = np.zeros((GPC, N_LOC), F32)
        goh2[batch[nodes] - c * GPC, li] = 1.0

        m = {
            "gidx": _wrap16(slot_pair, NSLOT),
            "xidx": _wrap16(xa_local, N_LOC),
            "pmask": np.repeat(
                slot_par.reshape(1, -1), 128, axis=0
            ).astype(np.uint8),
            "eaT": np.concatenate(
                [slot_ea.T, slot_bias.reshape(1, -1)], axis=0
            ).astype(F16),
            "ssch": ssch.reshape(128, NCHUNK * 256),
            "goh": goh,
            "goh2": goh2,
            "maskbias": ((goh - 1.0) * 1e30).astype(F32),
        }
        in_maps.append(m)

    # shared parameters
    conv_Wf = np.asarray(inputs["conv_Wf"], F32)
    conv_Ws = np.asarray(inputs["conv_Ws"], F32)
    conv_bf = np.asarray(inputs["conv_bf"], F32)
    conv_bs = np.asarray(inputs["conv_bs"], F32)
    conv_bn = np.asarray(inputs["conv_bn"], F32)

    # all message terms carry the HSC scale: fs_s = fs * HSC
    # src term: merged (= h*HSC) @ Wsrc  -> weights plain
    wsrc = np.concatenate(
        [
            np.concatenate([conv_Wf[l, H : 2 * H], conv_Ws[l, H : 2 * H]], 1)
            for l in range(L)
        ],
        axis=1,
    )  # [128, L*256]
    # dst term: p_fs = (h @ Wdst) * HSC -> fold HSC into Wdst
    wdst = np.concatenate(
        [
            np.concatenate([conv_Wf[l, :H], conv_Ws[l, :H]], 1)
            for l in range(L)
        ],
        axis=1,
    ) * HSC
    # edge term: ea @ (Wea * HSC) (+ bias * HSC)
    wea = np.concatenate(
        [
            np.concatenate(
                [
                    np.concatenate([conv_Wf[l, 2 * H :], conv_Ws[l, 2 * H :]], 1),
                    np.concatenate([conv_bf[l], conv_bs[l]]).reshape(1, -1),
                ],
                axis=0,
            )
            for l in range(L)
        ],
        axis=1,
    ) * HSC  # [51, L*256]
    wsrc_hi, wsrc_lo = _hilo(wsrc)
    wea_hi, wea_lo = _hilo(wea)
    convss = np.concatenate(
        [
            np.concatenate([_rep(sc), _rep(sh)], axis=1)
            for sc, sh in ((_bn_fold(conv_bn[l])) for l in range(L))
        ],
        axis=1,
    )  # [128, L*256]

    psc, psh = _bn_fold(np.asarray(inputs["proj_bn"], F32),
                        bias=np.asarray(inputs["proj_b"], F32))
    h1sc, h1sh = _bn_fold(np.asarray(inputs["head_bn1"], F32),
                          bias=np.asarray(inputs["head_b1"], F32))
    h2sc, h2sh = _bn_fold(np.asarray(inputs["head_bn2"], F32),
                          bias=np.asarray(inputs["head_b2"], F32))

    shared = {
        "emb": np.asarray(inputs["emb"], F32),
        "projW": np.asarray(inputs["proj_W"], F32),
        "projss": np.concatenate([_rep(psc), _rep(psh)], axis=1),
        "wsrchi": wsrc_hi,
        "wsrclo": wsrc_lo,
        "wdst": wdst.astype(F32),
        "weahi": wea_hi,
        "wealo": wea_lo,
        "convss": convss,
        "gatew1": np.asarray(inputs["gate_W1"], F32),
        "gateb1": _rep(np.asarray(inputs["gate_b1"], F32)),
        "gatew2": np.asarray(inputs["gate_W2"], F32),
        "gateb2": _rep(np.asarray(inputs["gate_b2"], F32).reshape(1)),
        "headw1": np.asarray(inputs["head_W1"], F32),
        "h1ss": np.concatenate([_rep(h1sc), _rep(h1sh)], axis=1),
        "headw2": np.asarray(inputs["head_W2"], F32),
        "h2ss": np.concatenate([_rep(h2sc), _rep(h2sh)], axis=1),
        "headw3": np.asarray(inputs["head_W3"], F32),
        "h3b": _rep(np.asarray(inputs["head_b3"], F32)),
        "headw4": np.asarray(inputs["head_W4"], F32),
        "h4b": _rep(np.asarray(inputs["head_b4"], F32).reshape(1)),
        "identf": np.eye(128, dtype=F32),
    }
    for m in in_maps:
        m.update(shared)
    return in_maps


# ---------------------------------------------------------------------------
# bass program
# ---------------------------------------------------------------------------

def _build():
    dt = mybir.dt
    nc = bacc.Bacc(num_devices=C)

    def par(name, shape, dtp):
        return nc.declare_dram_parameter(name, list(shape), dtp, isOutput=False)

    gidx_d = par("gidx", [128, NSLOT // 16], dt.int16)
    xidx_d = par("xidx", [128, N_LOC // 16], dt.int16)
    pmask_d = par("pmask", [128, NSLOT], dt.uint8)
    eaT_d = par("eaT", [ED + 1, NSLOT], dt.float16)
    ssch_d = par("ssch", [128, NCHUNK * 256], dt.float16)
    goh_d = par("goh", [128, NT * GPC], dt.float32)
    goh2_d = par("goh2", [GPC, N_LOC], dt.float32)
    maskbias_d = par("maskbias", [128, NT * GPC], dt.float32)
    emb_d = par("emb", [NEMB, H], dt.float32)
    projW_d = par("projW", [H, H], dt.float32)
    projss_d = par("projss", [128, 256], dt.float32)
    wsrchi_d = par("wsrchi", [H, L * 256], dt.float16)
    wsrclo_d = par("wsrclo", [H, L * 256], dt.float16)
    wdst_d = par("wdst", [H, L * 256], dt.float32)
    weahi_d = par("weahi", [ED + 1, L * 256], dt.float16)
    wealo_d = par("wealo", [ED + 1, L * 256], dt.float16)
    convss_d = par("convss", [128, L * 256], dt.float32)
    gatew1_d = par("gatew1", [H, H // 2], dt.float32)
    gateb1_d = par("gateb1", [128, H // 2], dt.float32)
    gatew2_d = par("gatew2", [H // 2, 1], dt.float32)
    gateb2_d = par("gateb2", [128, 1], dt.float32)
    headw1_d = par("headw1", [H, H], dt.float32)
    h1ss_d = par("h1ss", [128, 256], dt.float32)
    headw2_d = par("headw2", [H, H // 2], dt.float32)
    h2ss_d = par("h2ss", [128, 128], dt.float32)
    headw3_d = par("headw3", [H // 2, H // 4], dt.float32)
    h3b_d = par("h3b", [128, H // 4], dt.float32)
    headw4_d = par("headw4", [H // 4, 1], dt.float32)
    h4b_d = par("h4b", [128, 1], dt.float32)
    identf_d = par("identf", [128, 128], dt.float32)

    out_d = nc.declare_dram_parameter("out", [GPC, 1], dt.float32,
                                      isOutput=True)

    hstage = nc.dram_tensor("hstage", [N_LOC // 2, 256], dt.float16)
    hfull = [
        nc.dram_tensor(f"hfull{i}", [PAIRS, 256], dt.float16,
                       addr_space="Shared")
        for i in range(2)
    ]

    FT, F16T = dt.float32, dt.float16
    AF = mybir.ActivationFunctionType
    OP = mybir.AluOpType

    with tile.TileContext(nc) as tc:
        with (
            tc.tile_pool(name="const", bufs=1) as cpool,
            tc.tile_pool(name="state", bufs=1) as spool,
            tc.tile_pool(name="psA", bufs=2, space="PSUM") as psA,
            tc.tile_pool(name="psB", bufs=2, space="PSUM") as psB,
            tc.tile_pool(name="psT", bufs=2, space="PSUM") as psT,
            tc.tile_pool(name="psG", bufs=2, space="PSUM") as psG,
        ):
            # ---------------- resident tiles ----------------
            def load(pool, dram, shape, dtp, sfx=""):
                nm = f"c_{dram.name}{sfx}"
                t = pool.tile(shape, dtp, name=nm, tag=nm)
                nc.sync.dma_start(out=t[:], in_=dram[:])
                return t

            gidx_t = load(cpool, gidx_d, [128, NSLOT // 16], dt.int16)
            projW_t = load(cpool, projW_d, [H, H], FT)
            projss_t = load(cpool, projss_d, [128, 256], FT)
            wsrchi_t = load(cpool, wsrchi_d, [H, L * 256], F16T)
            wsrclo_t = load(cpool, wsrclo_d, [H, L * 256], F16T)
            wdst_t = load(cpool, wdst_d, [H, L * 256], FT)
            weahi_t = load(cpool, weahi_d, [ED + 1, L * 256], F16T)
            wealo_t = load(cpool, wealo_d, [ED + 1, L * 256], F16T)
            convss_t = load(cpool, convss_d, [128, L * 256], FT)
            identf_t = load(cpool, identf_d, [128, 128], FT)

            h_loc = spool.tile([128, NT, H], FT, tag="h_loc")
            pfs_hi = spool.tile([128, NT, 256], F16T, tag="pfs_hi")
            pfs_lo = spool.tile([128, NT, 256], F16T, tag="pfs_lo")
            h_bf = spool.tile([128, NT, H], F16T, tag="h_bf")

            for rep in range(_REPS):
                R = f"R{rep}_"

                # ---------------- embedding + projection ----------------
                with (
                    tc.tile_pool(name=f"{R}proj", bufs=2) as prpool,
                    tc.tile_pool(name=f"{R}projc", bufs=1) as prcpool,
                ):
                    xidx_t = load(prcpool, xidx_d, [128, N_LOC // 16],
                                  dt.int16, sfx=R)
                    TPG = 13  # node tiles per gather call
                    for g in range(NT // TPG):
                        h0 = prpool.tile([128, TPG, H], FT, tag="h0")
                        nc.gpsimd.dma_gather(
                            h0[:], emb_d[:],
                            xidx_t[:, g * (TPG * 8) : (g + 1) * (TPG * 8)],
                            TPG * 128, TPG * 128, H, single_packet=False,
                        )
                        for tt in range(TPG):
                            t = g * TPG + tt
                            pT = psT.tile([128, 128], FT, tag="tr",
                                          name=f"{R}prT{t}")
                            nc.tensor.transpose(pT[:], h0[:, tt, :],
                                                identf_t[:])
                            hT = prpool.tile([128, 128], FT, tag="hT32",
                                             name=f"{R}prh{t}")
                            nc.vector.tensor_copy(hT[:], pT[:])
                            pm = psB.tile([128, 256], FT, tag="pB",
                                          name=f"{R}prm{t}")
                            nc.tensor.matmul(pm[:, :H], hT[:], projW_t[:],
                                             start=True, stop=True)
                            t1 = prpool.tile([128, H], FT, tag="nupd",
                                             name=f"{R}pru{t}")
                            nc.vector.tensor_tensor(
                                out=t1[:], in0=pm[:, :H],
                                in1=projss_t[:, :128], op=OP.mult)
                            nc.vector.tensor_tensor(
                                out=t1[:], in0=t1[:], in1=projss_t[:, 128:],
                                op=OP.add)
                            sgp = prpool.tile([128, H], FT, tag="sgp",
                                              name=f"{R}prs{t}")
                            nc.scalar.activation(sgp[:], t1[:], AF.Sigmoid)
                            nc.vector.tensor_mul(out=h_loc[:, t, :],
                                                 in0=t1[:], in1=sgp[:])

                if _PHASE <= 1:
                    dbg = spool.tile([GPC, 1], FT, tag="dbg",
                                     name=f"{R}dbg1")
                    nc.vector.tensor_copy(dbg[:], h_loc[:GPC, 0, 0:1])
                    nc.sync.dma_start(out=out_d[:], in_=dbg[:])

                # ---------------- conv layers ----------------
                with (
                    tc.tile_pool(name=f"{R}gbuf", bufs=2) as gpool,
                    tc.tile_pool(name=f"{R}work", bufs=2) as wpool,
                    tc.tile_pool(name=f"{R}wk1", bufs=1) as w1pool,
                    tc.tile_pool(name=f"{R}small", bufs=3) as smpool,
                ):
                    TG = NT // GSPLIT  # ranges per AG group
                    for l in range(_L_RUN if _PHASE >= 2 else 0):
                        hf = hfull[l % 2]
                        # stage h as scaled fp16 + allgather, split in
                        # GSPLIT groups so AG(g) overlaps the tail of the
                        # previous layer's compute
                        for g in range(GSPLIT):
                            gt = slice(g * TG, (g + 1) * TG)
                            nc.vector.tensor_scalar_mul(
                                out=h_bf[:, gt, :]
                                .rearrange("p t h -> p (t h)"),
                                in0=h_loc[:, gt, :]
                                .rearrange("p t h -> p (t h)"),
                                scalar1=HSC)
                            nc.sync.dma_start(
                                out=hstage[g * ROWS_G : (g + 1) * ROWS_G, :]
                                .rearrange("n (two h) -> (n two) h", two=2)
                                .rearrange("(t p) h -> p t h", p=128),
                                in_=h_bf[:, gt, :],
                            )
                            nc.gpsimd.collective_compute(
                                "AllGather",
                                mybir.AluOpType.bypass,
                                replica_groups=[list(range(C))],
                                ins=[hstage[g * ROWS_G
                                            : (g + 1) * ROWS_G, :]],
                                outs=[hf[g * C * ROWS_G
                                         : (g + 1) * C * ROWS_G, :]],
                            )

                        # dst-side node projections pfs = (h@Wdst)*HSC, hi/lo
                        for t in range(NT):
                            pT = psT.tile([128, 128], FT, tag="tr",
                                          name=f"{R}pT_{l}_{t}")
                            nc.tensor.transpose(pT[:], h_loc[:, t, :],
                                                identf_t[:])
                            hTb = wpool.tile([128, 128], FT, tag="hTb",
                                             name=f"{R}hTb_{l}_{t}")
                            nc.vector.tensor_copy(hTb[:], pT[:])
                            pm = psB.tile([128, 256], FT, tag="pB",
                                          name=f"{R}pm_{l}_{t}")
                            nc.tensor.matmul(
                                pm[:], hTb[:],
                                wdst_t[:, l * 256 : (l + 1) * 256],
                                start=True, stop=True)
                            nc.vector.tensor_copy(pfs_hi[:, t, :], pm[:])
                            nc.vector.tensor_tensor(
                                out=pfs_lo[:, t, :], in0=pm[:],
                                in1=pfs_hi[:, t, :], op=OP.subtract)

                        aggr = {}
                        for b in range(NBLK):
                            bsl = slice(b * SLOT_B, (b + 1) * SLOT_B)
                            gb = gpool.tile([128, 2, SLOT_B], F16T, tag="gb",
                                            name=f"{R}gb_{l}_{b}")
                            if "nog" not in _ABL:
                                nc.gpsimd.dma_gather(
                                    gb[:], hf[:],
                                    gidx_t[:, b * (SLOT_B // 16)
                                           : (b + 1) * (SLOT_B // 16)],
                                    SLOT_B, SLOT_B, 256, transpose=True,
                                    single_packet=False,
                                )
                            mask = wpool.tile([128, SLOT_B], dt.uint8,
                                              tag="mask", name=f"{R}mk_{l}_{b}")
                            if "noea" not in _ABL:
                                nc.sync.dma_start(out=mask[:],
                                                  in_=pmask_d[:, bsl])
                            # merge even/odd half in place
                            if "nopred" not in _ABL:
                                nc.vector.copy_predicated(gb[:, 0, :],
                                                          mask[:],
                                                          gb[:, 1, :])
                            ea_t = wpool.tile([ED + 1, SLOT_B], F16T,
                                              tag="ea", name=f"{R}ea_{l}_{b}")
                            if "noea" not in _ABL:
                                nc.sync.dma_start(out=ea_t[:],
                                                  in_=eaT_d[:, bsl])
                            ssk = wpool.tile([128, CPB, 256], F16T,
                                             tag="ssk", name=f"{R}ssk_{l}_{b}")
                            if "nossk" not in _ABL:
                                nc.sync.dma_start(
                                    out=ssk[:],
                                    in_=ssch_d[:, b * CPB * 256
                                               : (b + 1) * CPB * 256])

                            # fs_s = fs*HSC per chunk; stashed fp16 as
                            # [128, 2(half), CPB, 128]
                            fsacc = wpool.tile([128, 2, CPB, 128], F16T,
                                               tag="fsacc",
                                               name=f"{R}fsa_{l}_{b}")
                            for j in range(CPB):
                                c = b * CPB + j
                                r = c // CPR
                                sl = slice(j * 128, (j + 1) * 128)
                                fs = psA.tile([128, 256], FT, tag="fs",
                                              name=f"{R}fs_{l}_{c}")
                                lsl = slice(l * 256, (l + 1) * 256)
                                if "nomm" in _ABL:
                                    continue
                                nc.tensor.matmul(fs[:], gb[:, 0, sl],
                                                 wsrchi_t[:, lsl],
                                                 start=True, stop=False)
                                nc.tensor.matmul(fs[:], gb[:, 0, sl],
                                                 wsrclo_t[:, lsl],
                                                 start=False, stop=False)
                                nc.tensor.matmul(fs[:], ea_t[:, sl],
                                                 weahi_t[:, lsl],
                                                 start=False, stop=False)
                                nc.tensor.matmul(fs[:], ea_t[:, sl],
                                                 wealo_t[:, lsl],
                                                 start=False, stop=False)
                                nc.tensor.matmul(fs[:], ssk[:, j, 128:256],
                                                 pfs_hi[:, r, :],
                                                 start=False, stop=False)
                                nc.tensor.matmul(fs[:], ssk[:, j, 128:256],
                                                 pfs_lo[:, r, :],
                                                 start=False, stop=True)
                                if "nocp" not in _ABL:
                                    nc.vector.tensor_copy(
                                        fsacc[:, :, j, :],
                                        fs[:].rearrange(
                                            "p (two h) -> p two h", two=2))

                            # block activations: sigmoid(f), sigmoid(-s), Ln
                            if "noact" in _ABL:
                                continue
                            sgf = w1pool.tile([128, CPB, 128], F16T,
                                              tag="sgf", name=f"{R}sgf_{l}_{b}")
                            nc.scalar.activation(
                                sgf[:].rearrange("p c h -> p (c h)"),
                                fsacc[:, 0, :, :]
                                .rearrange("p c h -> p (c h)"),
                                AF.Sigmoid, scale=1.0 / HSC)
                            sgc = wpool.tile([128, CPB, 128], F16T,
                                             tag="sgc", name=f"{R}sgc_{l}_{b}")
                            nc.scalar.activation(
                                sgc[:].rearrange("p c h -> p (c h)"),
                                fsacc[:, 1, :, :]
                                .rearrange("p c h -> p (c h)"),
                                AF.Sigmoid, scale=-1.0 / HSC)
                            spc = w1pool.tile([128, CPB, 128], F16T,
                                              tag="spc", name=f"{R}spc_{l}_{b}")
                            nc.vector.tensor_scalar_max(
                                out=spc[:].rearrange("p c h -> p (c h)"),
                                in0=sgc[:].rearrange("p c h -> p (c h)"),
                                scalar1=SIGC)
                            lnv = w1pool.tile([128, CPB, 128], FT,
                                              tag="lnv", name=f"{R}lnv_{l}_{b}")
                            nc.scalar.activation(
                                lnv[:].rearrange("p c h -> p (c h)"),
                                spc[:].rearrange("p c h -> p (c h)"), AF.Ln)
                            # sp_s = max(-ln(sigc)*HSC, s_s)
                            spv = wpool.tile([128, CPB, 128], F16T,
                                             tag="spv", name=f"{R}spv_{l}_{b}")
                            nc.vector.scalar_tensor_tensor(
                                out=spv[:].rearrange("p c h -> p (c h)"),
                                in0=lnv[:].rearrange("p c h -> p (c h)"),
                                scalar=-HSC,
                                in1=fsacc[:, 1, :, :]
                                .rearrange("p c h -> p (c h)"),
                                op0=OP.mult, op1=OP.max)
                            msgb = wpool.tile([128, CPB, 128], F16T,
                                              tag="msgb", name=f"{R}msg_{l}_{b}")
                            nc.vector.tensor_mul(
                                out=msgb[:].rearrange("p c h -> p (c h)"),
                                in0=sgf[:].rearrange("p c h -> p (c h)"),
                                in1=spv[:].rearrange("p c h -> p (c h)"))

                            for j in range(CPB):
                                c = b * CPB + j
                                r = c // CPR
                                if c % CPR == 0:
                                    aggr[r] = psG.tile([128, H], FT,
                                                       tag="aggr",
                                                       name=f"{R}aggr_{l}_{r}")
                                nc.tensor.matmul(
                                    aggr[r][:], ssk[:, j, 0:128],
                                    msgb[:, j, :],
                                    start=(c % CPR == 0),
                                    stop=(c % CPR == CPR - 1))
                                if c % CPR == CPR - 1:
                                    lss = convss_t[:, l * 256 : (l + 1) * 256]
                                    u = smpool.tile([128, H], FT, tag="nupd",
                                                    name=f"{R}u_{l}_{r}")
                                    nc.vector.scalar_tensor_tensor(
                                        out=u[:], in0=aggr[r][:],
                                        scalar=1.0 / HSC,
                                        in1=h_loc[:, r, :],
                                        op0=OP.mult, op1=OP.add)
                                    nc.vector.tensor_tensor(
                                        out=u[:], in0=u[:], in1=lss[:, :128],
                                        op=OP.mult)
                                    nc.vector.tensor_tensor(
                                        out=u[:], in0=u[:], in1=lss[:, 128:],
                                        op=OP.add)
                                    us = smpool.tile([128, H], FT,
                                                     tag="nsig",
                                                     name=f"{R}us_{l}_{r}")
                                    nc.scalar.activation(us[:], u[:],
                                                         AF.Sigmoid)
                                    nc.vector.tensor_mul(out=us[:], in0=u[:],
                                                         in1=us[:])
                                    nc.vector.tensor_tensor(
                                        out=h_loc[:, r, :], in0=us[:],
                                        in1=h_loc[:, r, :], op=OP.add)
                                    del aggr[r]

                if _PHASE == 2:
                    dbg2 = spool.tile([GPC, 1], FT, tag="dbg",
                                      name=f"{R}dbg2")
                    nc.vector.tensor_copy(dbg2[:], h_loc[:GPC, 0, 0:1])
                    nc.sync.dma_start(out=out_d[:], in_=dbg2[:])

                # ---------------- gate + pooling + head ----------------
                with (
                    tc.tile_pool(name=f"{R}poolc", bufs=1) as pcpool,
                    tc.tile_pool(name=f"{R}pools", bufs=3) as smpool,
                ):
                  if _PHASE >= 5:
                    goh_t = load(pcpool, goh_d, [128, NT * GPC], FT, sfx=R)
                    goh2_t = load(pcpool, goh2_d, [GPC, N_LOC], FT, sfx=R)
                    maskb_t = load(pcpool, maskbias_d, [128, NT * GPC], FT,
                                   sfx=R)
                    gatew1_t = load(pcpool, gatew1_d, [H, H // 2], FT, sfx=R)
                    gateb1_t = load(pcpool, gateb1_d, [128, H // 2], FT,
                                    sfx=R)
                    gatew2_t = load(pcpool, gatew2_d, [H // 2, 1], FT, sfx=R)
                    gateb2_t = load(pcpool, gateb2_d, [128, 1], FT, sfx=R)
                    headw1_t = load(pcpool, headw1_d, [H, H], FT, sfx=R)
                    h1ss_t = load(pcpool, h1ss_d, [128, 256], FT, sfx=R)
                    headw2_t = load(pcpool, headw2_d, [H, H // 2], FT, sfx=R)
                    h2ss_t = load(pcpool, h2ss_d, [128, 128], FT, sfx=R)
                    headw3_t = load(pcpool, headw3_d, [H // 2, H // 4], FT,
                                    sfx=R)
                    h3b_t = load(pcpool, h3b_d, [128, H // 4], FT, sfx=R)
                    headw4_t = load(pcpool, headw4_d, [H // 4, 1], FT, sfx=R)
                    h4b_t = load(pcpool, h4b_d, [128, 1], FT, sfx=R)

                    g_all = pcpool.tile([128, NT], FT, name=f"{R}g_all",
                                        tag="g_all")
                    runmax = pcpool.tile([128, GPC], FT, name=f"{R}runmax",
                                         tag="runmax")

                    # pass 1: per-node gate scores g + running per-graph max
                    for t in range(NT):
                        pT = psT.tile([128, 128], FT, tag="tr",
                                      name=f"{R}gT{t}")
                        nc.tensor.transpose(pT[:], h_loc[:, t, :],
                                            identf_t[:])
                        hT = smpool.tile([128, 128], FT, tag="hT32",
                                         name=f"{R}gh{t}")
                        nc.vector.tensor_copy(hT[:], pT[:])
                        g1 = psB.tile([128, 256], FT, tag="pB",
                                      name=f"{R}g1_{t}")
                        nc.tensor.matmul(g1[:, : H // 2], hT[:], gatew1_t[:],
                                         start=True, stop=True)
                        s1 = smpool.tile([128, H // 2], FT, tag="s1",
                                         name=f"{R}s1_{t}")
                        nc.vector.tensor_tensor(
                            out=s1[:], in0=g1[:, : H // 2], in1=gateb1_t[:],
                            op=OP.add)
                        s1s = smpool.tile([128, H // 2], FT, tag="s1s",
                                          name=f"{R}s1s_{t}")
                        nc.scalar.activation(s1s[:], s1[:], AF.Sigmoid)
                        nc.vector.tensor_mul(out=s1[:], in0=s1[:],
                                             in1=s1s[:])
                        pT2 = psT.tile([128, 128], FT, tag="tr",
                                       name=f"{R}gU{t}")
                        nc.tensor.transpose(pT2[: H // 2, :], s1[:],
                                            identf_t[:])
                        s1T = smpool.tile([H // 2, 128], FT, tag="s1T",
                                          name=f"{R}s1T_{t}")
                        nc.vector.tensor_copy(s1T[:], pT2[: H // 2, :])
                        g2 = psT.tile([128, 128], FT, tag="tr",
                                      name=f"{R}g2_{t}")
                        nc.tensor.matmul(g2[:, :1], s1T[:], gatew2_t[:],
                                         start=True, stop=True)
                        nc.vector.tensor_tensor(
                            out=g_all[:, t : t + 1], in0=g2[:, :1],
                            in1=gateb2_t[:], op=OP.add)
                        gm = smpool.tile([128, GPC], FT, tag="gm",
                                         name=f"{R}gm_{t}")
                        nc.vector.tensor_tensor(
                            out=gm[:],
                            in0=g_all[:, t : t + 1].to_broadcast([128, GPC]),
                            in1=goh_t[:, t * GPC : (t + 1) * GPC],
                            op=OP.mult)
                        nc.vector.tensor_tensor(
                            out=gm[:], in0=gm[:],
                            in1=maskb_t[:, t * GPC : (t + 1) * GPC],
                            op=OP.add)
                        if t == 0:
                            nc.vector.tensor_copy(runmax[:], gm[:])
                        else:
                            nc.vector.tensor_max(out=runmax[:],
                                                 in0=runmax[:], in1=gm[:])

                    # reduce running max across partitions -> gmax [GPC, 1]
                    pTm = psT.tile([128, 128], FT, tag="tr", name=f"{R}pTm")
                    nc.tensor.transpose(pTm[:GPC, :], runmax[:], identf_t[:])
                    rmT = smpool.tile([GPC, 128], FT, tag="rmT",
                                      name=f"{R}rmT")
                    nc.vector.tensor_copy(rmT[:], pTm[:GPC, :])
                    negmax = smpool.tile([GPC, 1], FT, tag="negmax",
                                         name=f"{R}negmax")
                    nc.vector.tensor_reduce(out=negmax[:], in_=rmT[:],
                                            axis=mybir.AxisListType.X,
                                            op=OP.max)
                    nc.vector.tensor_scalar_mul(out=negmax[:], in0=negmax[:],
                                                scalar1=-1.0)

                    # pass 2: e = exp(min(g - gmax[graph], 20)), pooled sums
                    pool_ps = psA.tile([GPC, H + 1], FT, tag="fs",
                                       name=f"{R}pool_ps")
                    for t in range(NT):
                        nK = psT.tile([128, 128], FT, tag="tr",
                                      name=f"{R}nK{t}")
                        nc.tensor.matmul(
                            nK[:, :1], goh2_t[:, t * 128 : (t + 1) * 128],
                            negmax[:], start=True, stop=True)
                        earg = smpool.tile([128, 1], FT, tag="earg",
                                           name=f"{R}ea2_{t}")
                        nc.vector.tensor_tensor(
                            out=earg[:], in0=g_all[:, t : t + 1],
                            in1=nK[:, :1], op=OP.add)
                        nc.vector.tensor_scalar_min(out=earg[:], in0=earg[:],
                                                    scalar1=20.0)
                        ecol = smpool.tile([128, 1], FT, tag="ecol",
                                           name=f"{R}ec_{t}")
                        nc.scalar.activation(ecol[:], earg[:], AF.Exp)
                        rhs = smpool.tile([128, H + 1], FT, tag="rhs",
                                          name=f"{R}rhs_{t}")
                        nc.vector.tensor_scalar(
                            out=rhs[:, :H], in0=h_loc[:, t, :],
                            scalar1=ecol[:], scalar2=None, op0=OP.mult)
                        nc.vector.tensor_copy(rhs[:, H : H + 1], ecol[:])
                        nc.tensor.matmul(
                            pool_ps[:], goh_t[:, t * GPC : (t + 1) * GPC],
                            rhs[:], start=(t == 0), stop=(t == NT - 1))

                    pooled_raw = smpool.tile([GPC, H + 1], FT, tag="praw",
                                             name=f"{R}praw")
                    nc.vector.tensor_copy(pooled_raw[:], pool_ps[:])
                    rec = smpool.tile([GPC, 1], FT, tag="rec", name=f"{R}rec")
                    nc.vector.reciprocal(rec[:], pooled_raw[:, H : H + 1])
                    pooled = smpool.tile([GPC, H], FT, tag="pooled",
                                         name=f"{R}pooled")
                    nc.vector.tensor_scalar(
                        out=pooled[:], in0=pooled_raw[:, :H], scalar1=rec[:],
                        scalar2=None, op0=OP.mult)

                    def head_mm(x, w, nin, nout, nm, ss=None, badd=None,
                                silu=True):
                        pT = psT.tile([128, 128], FT, tag="tr",
                                      name=f"{R}hT{nm}")
                        nc.tensor.transpose(pT[:nin, :GPC], x[:],
                                            identf_t[:GPC, :GPC])
                        xT = smpool.tile([128, GPC], FT, tag="xT",
                                         name=f"{R}xT{nm}")
                        nc.vector.tensor_copy(xT[:nin, :], pT[:nin, :GPC])
                        ym = psB.tile([128, 256], FT, tag="pB",
                                      name=f"{R}ym{nm}")
                        nc.tensor.matmul(ym[:GPC, :nout], xT[:nin, :], w[:],
                                         start=True, stop=True)
                        y = smpool.tile([GPC, nout], FT, tag=f"hd{nout}",
                                        name=f"{R}y{nm}")
                        if ss is not None:
                            nc.vector.tensor_tensor(
                                out=y[:], in0=ym[:GPC, :nout],
                                in1=ss[:GPC, :nout], op=OP.mult)
                            nc.vector.tensor_tensor(
                                out=y[:], in0=y[:],
                                in1=ss[:GPC, nout : 2 * nout], op=OP.add)
                        elif badd is not None:
                            nc.vector.tensor_tensor(
                                out=y[:], in0=ym[:GPC, :nout],
                                in1=badd[:GPC, :nout], op=OP.add)
                        else:
                            nc.vector.tensor_copy(y[:], ym[:GPC, :nout])
                        if silu:
                            ysig = smpool.tile([GPC, nout], FT,
                                               tag=f"hs{nout}",
                                               name=f"{R}ys{nm}")
                            nc.scalar.activation(ysig[:], y[:], AF.Sigmoid)
                            nc.vector.tensor_mul(out=y[:], in0=y[:],
                                                 in1=ysig[:])
                        return y

                    y1 = head_mm(pooled, headw1_t, H, H, "a", ss=h1ss_t)
                    y2 = head_mm(y1, headw2_t, H, H // 2, "b", ss=h2ss_t)
                    y3 = head_mm(y2, headw3_t, H // 2, H // 4, "c",
                                 badd=h3b_t)
                    y4 = head_mm(y3, headw4_t, H // 4, 1, "d", badd=h4b_t,
                                 silu=False)
                    nc.sync.dma_start(out=out_d[:], in_=y4[:])

    return nc


_NC_CACHE = None
_LAST_EXEC_NS = None


def kernel(**inputs) -> np.ndarray:
    global _NC_CACHE, _LAST_EXEC_NS
    in_maps = _prep(inputs)
    if _NC_CACHE is None:
        _NC_CACHE = _build()
        _NC_CACHE.finalize()
    trace = os.environ.get("KERNEL_TRACE", "0") == "1"
    res = run_bass_kernel_spmd(
        _NC_CACHE, in_maps, core_ids=list(range(C)), trace=trace
    )
    _LAST_EXEC_NS = res.exec_time_ns
    out = np.concatenate(
        [np.asarray(res.results[c]["out"]).reshape(GPC) for c in range(C)]
    )
    return out.astype(F32)


if __name__ == "__main__":
    import jax

    with jax.default_device(jax.devices("cpu")[0]):
        sys.path.insert(0, os.path.dirname(os.path.abspath(__file__)))
        import reference

        inp = {k: np.asarray(v) for k, v in reference.setup_inputs().items()}
    y = kernel(**inp)
    print("out[:8]:", y[:8])


# revision 15
# speedup vs baseline: 22.6786x; 1.4070x over previous
"""CGCNN regressor on 8 trn2 NeuronCores.

Sharding: graphs 32/core -> contiguous node blocks; edges live on dst's core.
Per core, nodes are permuted into 52 "ranges" of 128 (degree-balanced bin
packing, <=512 edges/range); each range owns 4 edge chunks of 128 slots.
Per layer: h (fp16, scaled by HSC) is AllGathered to a replicated pair-table
[26624, 256]; h[src] is fetched with one dma_gather(transpose=True) per block
(the pair trick keeps indices < 32768 int16) and the even/odd half is merged
in place with copy_predicated. Messages are computed as fp16 matmuls in
natural layout [slots, 256] with everything scaled by HSC so intermediates
fit fp16: src and edge_attr terms via hi/lo fp16 weight pairs, and the dst
term via a precomputed one-hot (sscT) against hi/lo fp16 dst projections.
Per gather block (13 chunks), sigmoid/softplus run as 3 large activations
(sigmoid on both halves, then Ln for a stable softplus), so activation-table
reloads happen twice per block instead of per chunk. Aggregation is a
one-hot matmul into [range,128] PSUM. Pool/head run on 32 graphs/core; host
concatenates the 8x[32] outputs.
"""

import os
import sys

import numpy as np

try:
    import concourse.bass as bass
except ImportError:  # grading env fallback
    sys.path.insert(0, "/opt/trn_rl_repo")
    import concourse.bass as bass

import concourse.mybir as mybir
import concourse.tile as tile
from concourse import bacc
from concourse.bass_utils import run_bass_kernel_spmd

F16 = np.float16
F32 = np.float32

# problem constants
N, E, H, ED, NG, NEMB, L = 50000, 200000, 128, 50, 256, 100, 6
C = 8               # cores
GPC = NG // C       # graphs per core
NT = 52             # node tiles (ranges) per core
N_LOC = NT * 128    # padded local nodes (6656)
CPR = 4             # chunks per range
NCHUNK = NT * CPR   # 208
NSLOT = NCHUNK * 128  # 26624 edge slots
CPB = 13            # chunks per gather block
NBLK = NCHUNK // CPB  # 16
SLOT_B = CPB * 128  # 1664 slots per block
PAIRS = C * N_LOC // 2  # 26624 pair rows in the replicated h table
GSPLIT = 2          # AllGather split: groups of NT/GSPLIT ranges
ROWS_G = N_LOC // 2 // GSPLIT  # pair rows per (core, group)
HSC = 1.0 / 16.0    # fp16 scale: h table, fs, p_fs, msg all carry HSC
SIGC = 6.1e-5       # sigmoid clamp before Ln (fp16 min normal)

_L_RUN = int(os.environ.get("KERNEL_LAYERS", str(L)))
_PHASE = int(os.environ.get("KERNEL_PHASE", "99"))  # 1=proj 2=+conv 99=all
_ABL = set(os.environ.get("KERNEL_ABL", "").split(","))  # timing ablations
_REPS = int(os.environ.get("KERNEL_REPS", "1"))  # repeat body in-NEFF


# ---------------------------------------------------------------------------
# host-side preprocessing
# ---------------------------------------------------------------------------

def _wrap16(idx, pad_to):
    """int16 index tensor in dma_gather layout: [128, pad_to//16],
    slot i -> row i%16, col i//16; replicated 8x down the partitions."""
    a = np.full(pad_to, 0, np.int16)
    a[: len(idx)] = idx.astype(np.int16)
    w = a.reshape(pad_to // 16, 16).T  # [16, pad/16]
    return np.tile(w, (8, 1)).copy()


def _bn_fold(p, bias=None):
    gamma, beta, mean, var = [np.asarray(x, np.float64) for x in p]
    scale = gamma / np.sqrt(var + 1e-5)
    shift = beta - mean * scale
    if bias is not None:
        shift = shift + np.asarray(bias, np.float64) * scale
    return scale.astype(F32), shift.astype(F32)


def _rep(row, parts=128):
    row = np.asarray(row, F32).reshape(1, -1)
    return np.repeat(row, parts, axis=0).copy()


def _hilo(x):
    x = np.asarray(x, F32)
    hi = x.astype(F16)
    lo = (x - hi.astype(F32)).astype(F16)
    return hi, lo


def _prep(inputs):
    x_atom = np.asarray(inputs["x_atom"]).astype(np.int64)
    ei = np.asarray(inputs["edge_index"]).astype(np.int64)
    ea = np.asarray(inputs["edge_attr"]).astype(F32)
    batch = np.asarray(inputs["batch"]).astype(np.int64)
    src, dst = ei[0], ei[1]

    node_start = np.searchsorted(batch, np.arange(0, NG + 1, GPC))
    deg = np.bincount(dst, minlength=N)

    # global node -> (core, local id); degree-balanced FFD into NT ranges/core
    lid = np.empty(N, np.int64)
    core_of = np.empty(N, np.int64)
    for c in range(C):
        s, e = node_start[c], node_start[c + 1]
        nodes = np.arange(s, e)
        assert len(nodes) <= N_LOC, f"core {c}: {len(nodes)} > {N_LOC}"
        order = nodes[np.argsort(-deg[nodes], kind="stable")]
        cap_n = np.full(NT, 128, np.int64)
        cap_e = np.full(NT, CPR * 128, np.int64)
        pos = np.zeros(NT, np.int64)
        for g in order:
            d = deg[g]
            cand = np.where((cap_n > 0) & (cap_e >= d))[0]
            assert len(cand), f"core {c}: range packing failed (deg {d})"
            r = cand[np.argmax(cap_e[cand])]
            lid[g] = r * 128 + pos[r]
            pos[r] += 1
            cap_n[r] -= 1
            cap_e[r] -= d
        core_of[s:e] = c

    # pair-row address in the group-major replicated table:
    # row = g*(C*ROWS_G) + core*ROWS_G + (p_local % ROWS_G), parity = lid&1
    p_local = lid >> 1
    grp = p_local // ROWS_G
    pair_row = grp * (C * ROWS_G) + core_of * ROWS_G + (p_local % ROWS_G)

    in_maps = []
    for c in range(C):
        s, e = node_start[c], node_start[c + 1]
        slot_pair = np.zeros(NSLOT, np.int64)
        slot_par = np.zeros(NSLOT, np.uint8)
        slot_dst = np.full(NSLOT, -1, np.int64)  # -1 = pad slot
        slot_ea = np.zeros((NSLOT, ED), F32)
        slot_bias = np.zeros(NSLOT, F32)

        emask = (dst >= s) & (dst < e)
        ce_src, ce_dst, ce_ea = src[emask], dst[emask], ea[emask]
        r_of_e = lid[ce_dst] // 128
        for r in range(NT):
            sel = np.where(r_of_e == r)[0]
            assert len(sel) <= CPR * 128, f"core {c} range {r}: {len(sel)}"
            base = r * CPR * 128
            sl = base + np.arange(len(sel))
            slot_pair[sl] = pair_row[ce_src[sel]]
            slot_par[sl] = (lid[ce_src[sel]] & 1).astype(np.uint8)
            slot_dst[sl] = lid[ce_dst[sel]] - r * 128
            slot_ea[sl] = ce_ea[sel]
            slot_bias[sl] = 1.0

        # precomputed one-hots: per chunk c, cols [0:128] = ssc (partition =
        # slot, col = dst row), cols [128:256] = sscT (partition = dst row,
        # col = slot)
        ssch = np.zeros((128, NCHUNK, 256), F16)
        sd = slot_dst.reshape(NCHUNK, 128)
        for ch in range(NCHUNK):
            valid = np.where(sd[ch] >= 0)[0]
            dcol = sd[ch][valid]
            ssch[valid, ch, dcol] = 1.0
            ssch[dcol, ch, 128 + valid] = 1.0

        # graph one-hot for pooling over local (permuted) node layout
        goh = np.zeros((128, NT * GPC), F32)
        xa_local = np.zeros(N_LOC, np.int64)
        nodes = np.arange(s, e)
        li = lid[nodes]
        xa_local[li] = x_atom[nodes]
        t_i, p_i = li // 128, li % 128
        goh[p_i, t_i * GPC + (batch[nodes] - c * GPC)] = 1.0

        goh2 = np.zeros((GPC, N_LOC), F32)
        goh2[batch[nodes] - c * GPC, li] = 1.0

        m = {
            "gidx": _wrap16(slot_pair, NSLOT),
            "xidx": _wrap16(xa_local, N_LOC),
            "pmask": np.repeat(
                slot_par.reshape(1, -1), 128, axis=0
            ).astype(np.uint8),
            "eaT": np.concatenate(
                [slot_ea.T, slot_bias.reshape(1, -1)], axis=0
            ).astype(F16),
            "ssch": ssch.reshape(128, NCHUNK * 256),
            "goh": goh,
            "goh2": goh2,
            "maskbias": ((goh - 1.0) * 1e30).astype(F32),
        }
        in_maps.append(m)

    # shared parameters
    conv_Wf = np.asarray(inputs["conv_Wf"], F32)
    conv_Ws = np.asarray(inputs["conv_Ws"], F32)
    conv_bf = np.asarray(inputs["conv_bf"], F32)
    conv_bs = np.asarray(inputs["conv_bs"], F32)
    conv_bn = np.asarray(inputs["conv_bn"], F32)

    # all message terms carry the HSC scale: fs_s = fs * HSC
    # src term: merged (= h*HSC) @ Wsrc  -> weights plain
    wsrc = np.concatenate(
        [
            np.concatenate([conv_Wf[l, H : 2 * H], conv_Ws[l, H : 2 * H]], 1)
            for l in range(L)
        ],
        axis=1,
    )  # [128, L*256]
    # dst term: p_fs = (h @ Wdst) * HSC -> fold HSC into Wdst
    wdst = np.concatenate(
        [
            np.concatenate([conv_Wf[l, :H], conv_Ws[l, :H]], 1)
            for l in range(L)
        ],
        axis=1,
    ) * HSC
    # edge term: ea @ (Wea * HSC) (+ bias * HSC)
    wea = np.concatenate(
        [
            np.concatenate(
                [
                    np.concatenate([conv_Wf[l, 2 * H :], conv_Ws[l, 2 * H :]], 1),
                    np.concatenate([conv_bf[l], conv_bs[l]]).reshape(1, -1),
                ],
                axis=0,
            )
            for l in range(L)
        ],
        axis=1,
    ) * HSC  # [51, L*256]
    wsrc_hi, wsrc_lo = _hilo(wsrc)
    wea_hi, wea_lo = _hilo(wea)
    convss = np.concatenate(
        [
            np.concatenate([_rep(sc), _rep(sh)], axis=1)
            for sc, sh in ((_bn_fold(conv_bn[l])) for l in range(L))
        ],
        axis=1,
    )  # [128, L*256]

    psc, psh = _bn_fold(np.asarray(inputs["proj_bn"], F32),
                        bias=np.asarray(inputs["proj_b"], F32))
    h1sc, h1sh = _bn_fold(np.asarray(inputs["head_bn1"], F32),
                          bias=np.asarray(inputs["head_b1"], F32))
    h2sc, h2sh = _bn_fold(np.asarray(inputs["head_bn2"], F32),
                          bias=np.asarray(inputs["head_b2"], F32))

    shared = {
        "emb": np.asarray(inputs["emb"], F32),
        "projW": np.asarray(inputs["proj_W"], F32),
        "projss": np.concatenate([_rep(psc), _rep(psh)], axis=1),
        "wsrchi": wsrc_hi,
        "wsrclo": wsrc_lo,
        "wdst": wdst.astype(F32),
        "weahi": wea_hi,
        "wealo": wea_lo,
        "convss": convss,
        "gatew1": np.asarray(inputs["gate_W1"], F32),
        "gateb1": _rep(np.asarray(inputs["gate_b1"], F32)),
        "gatew2": np.asarray(inputs["gate_W2"], F32),
        "gateb2": _rep(np.asarray(inputs["gate_b2"], F32).reshape(1)),
        "headw1": np.asarray(inputs["head_W1"], F32),
        "h1ss": np.concatenate([_rep(h1sc), _rep(h1sh)], axis=1),
        "headw2": np.asarray(inputs["head_W2"], F32),
        "h2ss": np.concatenate([_rep(h2sc), _rep(h2sh)], axis=1),
        "headw3": np.asarray(inputs["head_W3"], F32),
        "h3b": _rep(np.asarray(inputs["head_b3"], F32)),
        "headw4": np.asarray(inputs["head_W4"], F32),
        "h4b": _rep(np.asarray(inputs["head_b4"], F32).reshape(1)),
        "identf": np.eye(128, dtype=F32),
    }
    for m in in_maps:
        m.update(shared)
    return in_maps


# ---------------------------------------------------------------------------
# bass program
# ---------------------------------------------------------------------------

def _build():
    dt = mybir.dt
    nc = bacc.Bacc(num_devices=C)

    def par(name, shape, dtp):
        return nc.declare_dram_parameter(name, list(shape), dtp, isOutput=False)

    gidx_d = par("gidx", [128, NSLOT // 16], dt.int16)
    xidx_d = par("xidx", [128, N_LOC // 16], dt.int16)
    pmask_d = par("pmask", [128, NSLOT], dt.uint8)
    eaT_d = par("eaT", [ED + 1, NSLOT], dt.float16)
    ssch_d = par("ssch", [128, NCHUNK * 256], dt.float16)
    goh_d = par("goh", [128, NT * GPC], dt.float32)
    goh2_d = par("goh2", [GPC, N_LOC], dt.float32)
    maskbias_d = par("maskbias", [128, NT * GPC], dt.float32)
    emb_d = par("emb", [NEMB, H], dt.float32)
    projW_d = par("projW", [H, H], dt.float32)
    projss_d = par("projss", [128, 256], dt.float32)
    wsrchi_d = par("wsrchi", [H, L * 256], dt.float16)
    wsrclo_d = par("wsrclo", [H, L * 256], dt.float16)
    wdst_d = par("wdst", [H, L * 256], dt.float32)
    weahi_d = par("weahi", [ED + 1, L * 256], dt.float16)
    wealo_d = par("wealo", [ED + 1, L * 256], dt.float16)
    convss_d = par("convss", [128, L * 256], dt.float32)
    gatew1_d = par("gatew1", [H, H // 2], dt.float32)
    gateb1_d = par("gateb1", [128, H // 2], dt.float32)
    gatew2_d = par("gatew2", [H // 2, 1], dt.float32)
    gateb2_d = par("gateb2", [128, 1], dt.float32)
    headw1_d = par("headw1", [H, H], dt.float32)
    h1ss_d = par("h1ss", [128, 256], dt.float32)
    headw2_d = par("headw2", [H, H // 2], dt.float32)
    h2ss_d = par("h2ss", [128, 128], dt.float32)
    headw3_d = par("headw3", [H // 2, H // 4], dt.float32)
    h3b_d = par("h3b", [128, H // 4], dt.float32)
    headw4_d = par("headw4", [H // 4, 1], dt.float32)
    h4b_d = par("h4b", [128, 1], dt.float32)
    identf_d = par("identf", [128, 128], dt.float32)

    out_d = nc.declare_dram_parameter("out", [GPC, 1], dt.float32,
                                      isOutput=True)

    hstage = nc.dram_tensor("hstage", [N_LOC // 2, 256], dt.float16)
    hfull = [
        nc.dram_tensor(f"hfull{i}", [PAIRS, 256], dt.float16,
                       addr_space="Shared")
        for i in range(2)
    ]

    FT, F16T = dt.float32, dt.float16
    AF = mybir.ActivationFunctionType
    OP = mybir.AluOpType

    with tile.TileContext(nc) as tc:
        with (
            tc.tile_pool(name="const", bufs=1) as cpool,
            tc.tile_pool(name="state", bufs=1) as spool,
            tc.tile_pool(name="psA", bufs=2, space="PSUM") as psA,
            tc.tile_pool(name="psB", bufs=2, space="PSUM") as psB,
            tc.tile_pool(name="psT", bufs=2, space="PSUM") as psT,
            tc.tile_pool(name="psG", bufs=2, space="PSUM") as psG,
        ):
            # ---------------- resident tiles ----------------
            def load(pool, dram, shape, dtp, sfx=""):
                nm = f"c_{dram.name}{sfx}"
                t = pool.tile(shape, dtp, name=nm, tag=nm)
                nc.sync.dma_start(out=t[:], in_=dram[:])
                return t

            gidx_t = load(cpool, gidx_d, [128, NSLOT // 16], dt.int16)
            projW_t = load(cpool, projW_d, [H, H], FT)
            projss_t = load(cpool, projss_d, [128, 256], FT)
            wsrchi_t = load(cpool, wsrchi_d, [H, L * 256], F16T)
            wsrclo_t = load(cpool, wsrclo_d, [H, L * 256], F16T)
            wdst_t = load(cpool, wdst_d, [H, L * 256], FT)
            weahi_t = load(cpool, weahi_d, [ED + 1, L * 256], F16T)
            wealo_t = load(cpool, wealo_d, [ED + 1, L * 256], F16T)
            convss_t = load(cpool, convss_d, [128, L * 256], FT)
            identf_t = load(cpool, identf_d, [128, 128], FT)

            h_loc = spool.tile([128, NT, H], FT, tag="h_loc")
            pfs_hi = spool.tile([128, NT, 256], F16T, tag="pfs_hi")
            pfs_lo = spool.tile([128, NT, 256], F16T, tag="pfs_lo")
            h_bf = spool.tile([128, NT, H], F16T, tag="h_bf")

            for rep in range(_REPS):
                R = f"R{rep}_"

                # ---------------- embedding + projection ----------------
                with (
                    tc.tile_pool(name=f"{R}proj", bufs=2) as prpool,
                    tc.tile_pool(name=f"{R}projc", bufs=1) as prcpool,
                ):
                    xidx_t = load(prcpool, xidx_d, [128, N_LOC // 16],
                                  dt.int16, sfx=R)
                    TPG = 13  # node tiles per gather call
                    for g in range(NT // TPG):
                        h0 = prpool.tile([128, TPG, H], FT, tag="h0")
                        nc.gpsimd.dma_gather(
                            h0[:], emb_d[:],
                            xidx_t[:, g * (TPG * 8) : (g + 1) * (TPG * 8)],
                            TPG * 128, TPG * 128, H, single_packet=False,
                        )
                        for tt in range(TPG):
                            t = g * TPG + tt
                            pT = psT.tile([128, 128], FT, tag="tr",
                                          name=f"{R}prT{t}")
                            nc.tensor.transpose(pT[:], h0[:, tt, :],
                                                identf_t[:])
                            hT = prpool.tile([128, 128], FT, tag="hT32",
                                             name=f"{R}prh{t}")
                            nc.vector.tensor_copy(hT[:], pT[:])
                            pm = psB.tile([128, 256], FT, tag="pB",
                                          name=f"{R}prm{t}")
                            nc.tensor.matmul(pm[:, :H], hT[:], projW_t[:],
                                             start=True, stop=True)
                            t1 = prpool.tile([128, H], FT, tag="nupd",
                                             name=f"{R}pru{t}")
                            nc.vector.tensor_tensor(
                                out=t1[:], in0=pm[:, :H],
                                in1=projss_t[:, :128], op=OP.mult)
                            nc.vector.tensor_tensor(
                                out=t1[:], in0=t1[:], in1=projss_t[:, 128:],
                                op=OP.add)
                            sgp = prpool.tile([128, H], FT, tag="sgp",
                                              name=f"{R}prs{t}")
                            nc.scalar.activation(sgp[:], t1[:], AF.Sigmoid)
                            nc.vector.tensor_mul(out=h_loc[:, t, :],
                                                 in0=t1[:], in1=sgp[:])

                if _PHASE <= 1:
                    dbg = spool.tile([GPC, 1], FT, tag="dbg",
                                     name=f"{R}dbg1")
                    nc.vector.tensor_copy(dbg[:], h_loc[:GPC, 0, 0:1])
                    nc.sync.dma_start(out=out_d[:], in_=dbg[:])

                # ---------------- conv layers ----------------
                with (
                    tc.tile_pool(name=f"{R}gbuf", bufs=2) as gpool,
                    tc.tile_pool(name=f"{R}work", bufs=2) as wpool,
                    tc.tile_pool(name=f"{R}wk1", bufs=1) as w1pool,
                    tc.tile_pool(name=f"{R}small", bufs=3) as smpool,
                ):
                    TG = NT // GSPLIT  # ranges per AG group
                    for l in range(_L_RUN if _PHASE >= 2 else 0):
                        hf = hfull[l % 2]
                        # stage h as scaled fp16 + allgather, split in
                        # GSPLIT groups so AG(g) overlaps the tail of the
                        # previous layer's compute
                        for g in range(GSPLIT):
                            gt = slice(g * TG, (g + 1) * TG)
                            nc.vector.tensor_scalar_mul(
                                out=h_bf[:, gt, :]
                                .rearrange("p t h -> p (t h)"),
                                in0=h_loc[:, gt, :]
                                .rearrange("p t h -> p (t h)"),
                                scalar1=HSC)
                            nc.sync.dma_start(
                                out=hstage[g * ROWS_G : (g + 1) * ROWS_G, :]
                                .rearrange("n (two h) -> (n two) h", two=2)
                                .rearrange("(t p) h -> p t h", p=128),
                                in_=h_bf[:, gt, :],
                            )
                            nc.gpsimd.collective_compute(
                                "AllGather",
                                mybir.AluOpType.bypass,
                                replica_groups=[list(range(C))],
                                ins=[hstage[g * ROWS_G
                                            : (g + 1) * ROWS_G, :]],
                                outs=[hf[g * C * ROWS_G
                                         : (g + 1) * C * ROWS_G, :]],
                            )

                        # dst-side node projections pfs = (h@Wdst)*HSC, hi/lo
                        for t in range(NT):
                            pT = psT.tile([128, 128], FT, tag="tr",
                                          name=f"{R}pT_{l}_{t}")
                            nc.tensor.transpose(pT[:], h_loc[:, t, :],
                                                identf_t[:])
                            hTb = wpool.tile([128, 128], FT, tag="hTb",
                                             name=f"{R}hTb_{l}_{t}")
                            nc.vector.tensor_copy(hTb[:], pT[:])
                            pm = psB.tile([128, 256], FT, tag="pB",
                                          name=f"{R}pm_{l}_{t}")
                            nc.tensor.matmul(
                                pm[:], hTb[:],
                                wdst_t[:, l * 256 : (l + 1) * 256],
                                start=True, stop=True)
                            nc.vector.tensor_copy(pfs_hi[:, t, :], pm[:])
                            nc.vector.tensor_tensor(
                                out=pfs_lo[:, t, :], in0=pm[:],
                                in1=pfs_hi[:, t, :], op=OP.subtract)

                        aggr = {}
                        for b in range(NBLK):
                            bsl = slice(b * SLOT_B, (b + 1) * SLOT_B)
                            gb = gpool.tile([128, 2, SLOT_B], F16T, tag="gb",
                                            name=f"{R}gb_{l}_{b}")
                            if "nog" not in _ABL:
                                nc.gpsimd.dma_gather(
                                    gb[:], hf[:],
                                    gidx_t[:, b * (SLOT_B // 16)
                                           : (b + 1) * (SLOT_B // 16)],
                                    SLOT_B, SLOT_B, 256, transpose=True,
                                    single_packet=False,
                                )
                            mask = wpool.tile([128, SLOT_B], dt.uint8,
                                              tag="mask", name=f"{R}mk_{l}_{b}")
                            if "noea" not in _ABL:
                                nc.sync.dma_start(out=mask[:],
                                                  in_=pmask_d[:, bsl])
                            # merge even/odd half in place
                            if "nopred" not in _ABL:
                                nc.vector.copy_predicated(gb[:, 0, :],
                                                          mask[:],
                                                          gb[:, 1, :])
                            ea_t = wpool.tile([ED + 1, SLOT_B], F16T,
                                              tag="ea", name=f"{R}ea_{l}_{b}")
                            if "noea" not in _ABL:
                                nc.sync.dma_start(out=ea_t[:],
                                                  in_=eaT_d[:, bsl])
                            ssk = wpool.tile([128, CPB, 256], F16T,
                                             tag="ssk", name=f"{R}ssk_{l}_{b}")
                            if "nossk" not in _ABL:
                                nc.sync.dma_start(
                                    out=ssk[:],
                                    in_=ssch_d[:, b * CPB * 256
                                               : (b + 1) * CPB * 256])

                            # fs_s = fs*HSC per chunk; stashed fp16 as
                            # [128, 2(half), CPB, 128]
                            fsacc = wpool.tile([128, 2, CPB, 128], F16T,
                                               tag="fsacc",
                                               name=f"{R}fsa_{l}_{b}")
                            for j in range(CPB):
                                c = b * CPB + j
                                r = c // CPR
                                sl = slice(j * 128, (j + 1) * 128)
                                fs = psA.tile([128, 256], FT, tag="fs",
                                              name=f"{R}fs_{l}_{c}")
                                lsl = slice(l * 256, (l + 1) * 256)
                                if "nomm" in _ABL:
                                    continue
                                nc.tensor.matmul(fs[:], gb[:, 0, sl],
                                                 wsrchi_t[:, lsl],
                                                 start=True, stop=False)
                                nc.tensor.matmul(fs[:], gb[:, 0, sl],
                                                 wsrclo_t[:, lsl],
                                                 start=False, stop=False)
                                nc.tensor.matmul(fs[:], ea_t[:, sl],
                                                 weahi_t[:, lsl],
                                                 start=False, stop=False)
                                if "nowl" not in _ABL:
                                    nc.tensor.matmul(fs[:], ea_t[:, sl],
                                                     wealo_t[:, lsl],
                                                     start=False, stop=False)
                                nc.tensor.matmul(fs[:], ssk[:, j, 128:256],
                                                 pfs_hi[:, r, :],
                                                 start=False, stop=False)
                                nc.tensor.matmul(fs[:], ssk[:, j, 128:256],
                                                 pfs_lo[:, r, :],
                                                 start=False, stop=True)
                                if "nocp" not in _ABL:
                                    nc.vector.tensor_copy(
                                        fsacc[:, :, j, :],
                                        fs[:].rearrange(
                                            "p (two h) -> p two h", two=2))

                            # block activations: sigmoid(f), sigmoid(-s), Ln
                            if "noact" in _ABL:
                                continue
                            sgf = w1pool.tile([128, CPB, 128], F16T,
                                              tag="sgf", name=f"{R}sgf_{l}_{b}")
                            nc.scalar.activation(
                                sgf[:].rearrange("p c h -> p (c h)"),
                                fsacc[:, 0, :, :]
                                .rearrange("p c h -> p (c h)"),
                                AF.Sigmoid, scale=1.0 / HSC)
                            sgc = wpool.tile([128, CPB, 128], F16T,
                                             tag="sgc", name=f"{R}sgc_{l}_{b}")
                            nc.scalar.activation(
                                sgc[:].rearrange("p c h -> p (c h)"),
                                fsacc[:, 1, :, :]
                                .rearrange("p c h -> p (c h)"),
                                AF.Sigmoid, scale=-1.0 / HSC)
                            spc = w1pool.tile([128, CPB, 128], F16T,
                                              tag="spc", name=f"{R}spc_{l}_{b}")
                            nc.vector.tensor_scalar_max(
                                out=spc[:].rearrange("p c h -> p (c h)"),
                                in0=sgc[:].rearrange("p c h -> p (c h)"),
                                scalar1=SIGC)
                            lnv = w1pool.tile([128, CPB, 128], FT,
                                              tag="lnv", name=f"{R}lnv_{l}_{b}")
                            nc.scalar.activation(
                                lnv[:].rearrange("p c h -> p (c h)"),
                                spc[:].rearrange("p c h -> p (c h)"), AF.Ln)
                            # sp_s = max(-ln(sigc)*HSC, s_s)
                            spv = wpool.tile([128, CPB, 128], F16T,
                                             tag="spv", name=f"{R}spv_{l}_{b}")
                            nc.vector.scalar_tensor_tensor(
                                out=spv[:].rearrange("p c h -> p (c h)"),
                                in0=lnv[:].rearrange("p c h -> p (c h)"),
                                scalar=-HSC,
                                in1=fsacc[:, 1, :, :]
                                .rearrange("p c h -> p (c h)"),
                                op0=OP.mult, op1=OP.max)
                            msgb = wpool.tile([128, CPB, 128], F16T,
                                              tag="msgb", name=f"{R}msg_{l}_{b}")
                            nc.vector.tensor_mul(
                                out=msgb[:].rearrange("p c h -> p (c h)"),
                                in0=sgf[:].rearrange("p c h -> p (c h)"),
                                in1=spv[:].rearrange("p c h -> p (c h)"))

                            for j in range(CPB):
                                c = b * CPB + j
                                r = c // CPR
                                if c % CPR == 0:
                                    aggr[r] = psG.tile([128, H], FT,
                                                       tag="aggr",
                                                       name=f"{R}aggr_{l}_{r}")
                                nc.tensor.matmul(
                                    aggr[r][:], ssk[:, j, 0:128],
                                    msgb[:, j, :],
                                    start=(c % CPR == 0),
                                    stop=(c % CPR == CPR - 1))
                                if c % CPR == CPR - 1:
                                    lss = convss_t[:, l * 256 : (l + 1) * 256]
                                    u = smpool.tile([128, H], FT, tag="nupd",
                                                    name=f"{R}u_{l}_{r}")
                                    nc.vector.scalar_tensor_tensor(
                                        out=u[:], in0=aggr[r][:],
                                        scalar=1.0 / HSC,
                                        in1=h_loc[:, r, :],
                                        op0=OP.mult, op1=OP.add)
                                    nc.vector.tensor_tensor(
                                        out=u[:], in0=u[:], in1=lss[:, :128],
                                        op=OP.mult)
                                    nc.vector.tensor_tensor(
                                        out=u[:], in0=u[:], in1=lss[:, 128:],
                                        op=OP.add)
                                    us = smpool.tile([128, H], FT,
                                                     tag="nsig",
                                                     name=f"{R}us_{l}_{r}")
                                    nc.scalar.activation(us[:], u[:],
                                                         AF.Sigmoid)
                                    nc.vector.tensor_mul(out=us[:], in0=u[:],
                                                         in1=us[:])
                                    nc.vector.tensor_tensor(
                                        out=h_loc[:, r, :], in0=us[:],
                                        in1=h_loc[:, r, :], op=OP.add)
                                    del aggr[r]

                if _PHASE == 2:
                    dbg2 = spool.tile([GPC, 1], FT, tag="dbg",
                                      name=f"{R}dbg2")
                    nc.vector.tensor_copy(dbg2[:], h_loc[:GPC, 0, 0:1])
                    nc.sync.dma_start(out=out_d[:], in_=dbg2[:])

                # ---------------- gate + pooling + head ----------------
                with (
                    tc.tile_pool(name=f"{R}poolc", bufs=1) as pcpool,
                    tc.tile_pool(name=f"{R}pools", bufs=3) as smpool,
                ):
                  if _PHASE >= 5:
                    goh_t = load(pcpool, goh_d, [128, NT * GPC], FT, sfx=R)
                    goh2_t = load(pcpool, goh2_d, [GPC, N_LOC], FT, sfx=R)
                    maskb_t = load(pcpool, maskbias_d, [128, NT * GPC], FT,
                                   sfx=R)
                    gatew1_t = load(pcpool, gatew1_d, [H, H // 2], FT, sfx=R)
                    gateb1_t = load(pcpool, gateb1_d, [128, H // 2], FT,
                                    sfx=R)
                    gatew2_t = load(pcpool, gatew2_d, [H // 2, 1], FT, sfx=R)
                    gateb2_t = load(pcpool, gateb2_d, [128, 1], FT, sfx=R)
                    headw1_t = load(pcpool, headw1_d, [H, H], FT, sfx=R)
                    h1ss_t = load(pcpool, h1ss_d, [128, 256], FT, sfx=R)
                    headw2_t = load(pcpool, headw2_d, [H, H // 2], FT, sfx=R)
                    h2ss_t = load(pcpool, h2ss_d, [128, 128], FT, sfx=R)
                    headw3_t = load(pcpool, headw3_d, [H // 2, H // 4], FT,
                                    sfx=R)
                    h3b_t = load(pcpool, h3b_d, [128, H // 4], FT, sfx=R)
                    headw4_t = load(pcpool, headw4_d, [H // 4, 1], FT, sfx=R)
                    h4b_t = load(pcpool, h4b_d, [128, 1], FT, sfx=R)

                    g_all = pcpool.tile([128, NT], FT, name=f"{R}g_all",
                                        tag="g_all")
                    runmax = pcpool.tile([128, GPC], FT, name=f"{R}runmax",
                                         tag="runmax")

                    # pass 1: per-node gate scores g + running per-graph max
                    for t in range(NT):
                        pT = psT.tile([128, 128], FT, tag="tr",
                                      name=f"{R}gT{t}")
                        nc.tensor.transpose(pT[:], h_loc[:, t, :],
                                            identf_t[:])
                        hT = smpool.tile([128, 128], FT, tag="hT32",
                                         name=f"{R}gh{t}")
                        nc.vector.tensor_copy(hT[:], pT[:])
                        g1 = psB.tile([128, 256], FT, tag="pB",
                                      name=f"{R}g1_{t}")
                        nc.tensor.matmul(g1[:, : H // 2], hT[:], gatew1_t[:],
                                         start=True, stop=True)
                        s1 = smpool.tile([128, H // 2], FT, tag="s1",
                                         name=f"{R}s1_{t}")
                        nc.vector.tensor_tensor(
                            out=s1[:], in0=g1[:, : H // 2], in1=gateb1_t[:],
                            op=OP.add)
                        s1s = smpool.tile([128, H // 2], FT, tag="s1s",
                                          name=f"{R}s1s_{t}")
                        nc.scalar.activation(s1s[:], s1[:], AF.Sigmoid)
                        nc.vector.tensor_mul(out=s1[:], in0=s1[:],
                                             in1=s1s[:])
                        pT2 = psT.tile([128, 128], FT, tag="tr",
                                       name=f"{R}gU{t}")
                        nc.tensor.transpose(pT2[: H // 2, :], s1[:],
                                            identf_t[:])
                        s1T = smpool.tile([H // 2, 128], FT, tag="s1T",
                                          name=f"{R}s1T_{t}")
                        nc.vector.tensor_copy(s1T[:], pT2[: H // 2, :])
                        g2 = psT.tile([128, 128], FT, tag="tr",
                                      name=f"{R}g2_{t}")
                        nc.tensor.matmul(g2[:, :1], s1T[:], gatew2_t[:],
                                         start=True, stop=True)
                        nc.vector.tensor_tensor(
                            out=g_all[:, t : t + 1], in0=g2[:, :1],
                            in1=gateb2_t[:], op=OP.add)
                        gm = smpool.tile([128, GPC], FT, tag="gm",
                                         name=f"{R}gm_{t}")
                        nc.vector.tensor_tensor(
                            out=gm[:],
                            in0=g_all[:, t : t + 1].to_broadcast([128, GPC]),
                            in1=goh_t[:, t * GPC : (t + 1) * GPC],
                            op=OP.mult)
                        nc.vector.tensor_tensor(
                            out=gm[:], in0=gm[:],
                            in1=maskb_t[:, t * GPC : (t + 1) * GPC],
                            op=OP.add)
                        if t == 0:
                            nc.vector.tensor_copy(runmax[:], gm[:])
                        else:
                            nc.vector.tensor_max(out=runmax[:],
                                                 in0=runmax[:], in1=gm[:])

                    # reduce running max across partitions -> gmax [GPC, 1]
                    pTm = psT.tile([128, 128], FT, tag="tr", name=f"{R}pTm")
                    nc.tensor.transpose(pTm[:GPC, :], runmax[:], identf_t[:])
                    rmT = smpool.tile([GPC, 128], FT, tag="rmT",
                                      name=f"{R}rmT")
                    nc.vector.tensor_copy(rmT[:], pTm[:GPC, :])
                    negmax = smpool.tile([GPC, 1], FT, tag="negmax",
                                         name=f"{R}negmax")
                    nc.vector.tensor_reduce(out=negmax[:], in_=rmT[:],
                                            axis=mybir.AxisListType.X,
                                            op=OP.max)
                    nc.vector.tensor_scalar_mul(out=negmax[:], in0=negmax[:],
                                                scalar1=-1.0)

                    # pass 2: e = exp(min(g - gmax[graph], 20)), pooled sums
                    pool_ps = psA.tile([GPC, H + 1], FT, tag="fs",
                                       name=f"{R}pool_ps")
                    for t in range(NT):
                        nK = psT.tile([128, 128], FT, tag="tr",
                                      name=f"{R}nK{t}")
                        nc.tensor.matmul(
                            nK[:, :1], goh2_t[:, t * 128 : (t + 1) * 128],
                            negmax[:], start=True, stop=True)
                        earg = smpool.tile([128, 1], FT, tag="earg",
                                           name=f"{R}ea2_{t}")
                        nc.vector.tensor_tensor(
                            out=earg[:], in0=g_all[:, t : t + 1],
                            in1=nK[:, :1], op=OP.add)
                        nc.vector.tensor_scalar_min(out=earg[:], in0=earg[:],
                                                    scalar1=20.0)
                        ecol = smpool.tile([128, 1], FT, tag="ecol",
                                           name=f"{R}ec_{t}")
                        nc.scalar.activation(ecol[:], earg[:], AF.Exp)
                        rhs = smpool.tile([128, H + 1], FT, tag="rhs",
                                          name=f"{R}rhs_{t}")
                        nc.vector.tensor_scalar(
                            out=rhs[:, :H], in0=h_loc[:, t, :],
                            scalar1=ecol[:], scalar2=None, op0=OP.mult)
                        nc.vector.tensor_copy(rhs[:, H : H + 1], ecol[:])
                        nc.tensor.matmul(
                            pool_ps[:], goh_t[:, t * GPC : (t + 1) * GPC],
                            rhs[:], start=(t == 0), stop=(t == NT - 1))

                    pooled_raw = smpool.tile([GPC, H + 1], FT, tag="praw",
                                             name=f"{R}praw")
                    nc.vector.tensor_copy(pooled_raw[:], pool_ps[:])
                    rec = smpool.tile([GPC, 1], FT, tag="rec", name=f"{R}rec")
                    nc.vector.reciprocal(rec[:], pooled_raw[:, H : H + 1])
                    pooled = smpool.tile([GPC, H], FT, tag="pooled",
                                         name=f"{R}pooled")
                    nc.vector.tensor_scalar(
                        out=pooled[:], in0=pooled_raw[:, :H], scalar1=rec[:],
                        scalar2=None, op0=OP.mult)

                    def head_mm(x, w, nin, nout, nm, ss=None, badd=None,
                                silu=True):
                        pT = psT.tile([128, 128], FT, tag="tr",
                                      name=f"{R}hT{nm}")
                        nc.tensor.transpose(pT[:nin, :GPC], x[:],
                                            identf_t[:GPC, :GPC])
                        xT = smpool.tile([128, GPC], FT, tag="xT",
                                         name=f"{R}xT{nm}")
                        nc.vector.tensor_copy(xT[:nin, :], pT[:nin, :GPC])
                        ym = psB.tile([128, 256], FT, tag="pB",
                                      name=f"{R}ym{nm}")
                        nc.tensor.matmul(ym[:GPC, :nout], xT[:nin, :], w[:],
                                         start=True, stop=True)
                        y = smpool.tile([GPC, nout], FT, tag=f"hd{nout}",
                                        name=f"{R}y{nm}")
                        if ss is not None:
                            nc.vector.tensor_tensor(
                                out=y[:], in0=ym[:GPC, :nout],
                                in1=ss[:GPC, :nout], op=OP.mult)
                            nc.vector.tensor_tensor(
                                out=y[:], in0=y[:],
                                in1=ss[:GPC, nout : 2 * nout], op=OP.add)
                        elif badd is not None:
                            nc.vector.tensor_tensor(
                                out=y[:], in0=ym[:GPC, :nout],
                                in1=badd[:GPC, :nout], op=OP.add)
                        else:
                            nc.vector.tensor_copy(y[:], ym[:GPC, :nout])
                        if silu:
                            ysig = smpool.tile([GPC, nout], FT,
                                               tag=f"hs{nout}",
                                               name=f"{R}ys{nm}")
                            nc.scalar.activation(ysig[:], y[:], AF.Sigmoid)
                            nc.vector.tensor_mul(out=y[:], in0=y[:],
                                                 in1=ysig[:])
                        return y

                    y1 = head_mm(pooled, headw1_t, H, H, "a", ss=h1ss_t)
                    y2 = head_mm(y1, headw2_t, H, H // 2, "b", ss=h2ss_t)
                    y3 = head_mm(y2, headw3_t, H // 2, H // 4, "c",
                                 badd=h3b_t)
                    y4 = head_mm(y3, headw4_t, H // 4, 1, "d", badd=h4b_t,
                                 silu=False)
                    nc.sync.dma_start(out=out_d[:], in_=y4[:])

    return nc


_NC_CACHE = None
_LAST_EXEC_NS = None


def kernel(**inputs) -> np.ndarray:
    global _NC_CACHE, _LAST_EXEC_NS
    in_maps = _prep(inputs)
    if _NC_CACHE is None:
        _NC_CACHE = _build()
        _NC_CACHE.finalize()
    trace = os.environ.get("KERNEL_TRACE", "0") == "1"
    res = run_bass_kernel_spmd(
        _NC_CACHE, in_maps, core_ids=list(range(C)), trace=trace
    )
    _LAST_EXEC_NS = res.exec_time_ns
    out = np.concatenate(
        [np.asarray(res.results[c]["out"]).reshape(GPC) for c in range(C)]
    )
    return out.astype(F32)


if __name__ == "__main__":
    import jax

    with jax.default_device(jax.devices("cpu")[0]):
        sys.path.insert(0, os.path.dirname(os.path.abspath(__file__)))
        import reference

        inp = {k: np.asarray(v) for k, v in reference.setup_inputs().items()}
    y = kernel(**inp)
    print("out[:8]:", y[:8])
